# revision 13
# baseline (speedup 1.0000x reference)
"""DeepseekV2 MLA attention on 8 TRN2 NeuronCores (Bass/Tile), v2.

Strategy (tensor-parallel over heads, 2 heads/core), changes vs v1:
  - KV path is two-stage: stage A computes the full 576-dim kv_a latent for
    this core's own 256 tokens (dedicated xta input slice), one AllGather
    (~21us) shares the latent + per-token kv ssq across cores, stage B
    (latent @ kv_b, 512-contraction) replaces the fused wk/wv/wkpe
    projections (2048-contraction): ~120k fewer PE columns per core.
  - V comes out of stage B feature-major; PE-transposed into natural [s, dv]
    tiles with the RMSNorm scale fused into the per-partition copy.
  - The q-ssq pass shares the single xt stream with the wq projections
    (xt streamed once, not twice); its AllReduce is split in two so early
    attention columns unblock sooner.
  - All reciprocals via reciprocal_approx_fast (5x; the [1,512]
    single-partition reciprocals were 3.3us each on DVE).
  - RoPE pairs host-permuted to block layout; softmax scale folded into Wq;
    RMSNorm ln weights folded into the following projections (exact).
  - Attention in S^T[s,t] layout: scores via PE (k stationary), exp on ACT
    (no max subtraction; |scores| ~ O(1)), causal masking via additive mask
    on diagonal tiles, denominator via ones-vector matmul, PV with natural-
    layout V stationary.  o_proj row-parallel, host sums 8 partials.
"""

import os
import sys

import numpy as np

for _p in ("/opt/trn_rl_repo",):
    if _p not in sys.path and os.path.isdir(_p):
        sys.path.insert(0, _p)

import ml_dtypes  # noqa: E402

BF16 = ml_dtypes.bfloat16

H = 16
D_NOPE = 128
D_ROPE = 64
D_V = 128
KV_RANK = 512
Q_RANK = 1536
HIDDEN = 2048
T = 2048
EPS = 1e-6
QK_DIM = D_NOPE + D_ROPE
SCALE = QK_DIM ** -0.5
ROPE_BASE = 10000.0

N_CORES = 8
HPC = H // N_CORES          # heads per core = 2
TCOL = 512                  # moving-operand width
TCA = 256                   # stage-A token slice per core
NJ = T // TCOL              # 4 t-column blocks
NK = HIDDEN // 128          # 16 contraction chunks
NL = KV_RANK // 128         # 4 latent chunks
NS = T // 128               # 16 key tiles
NEG = -1.0e4                # causal mask additive value

_CACHE = {}


def _build_program():
    import concourse.bass as bass  # noqa: F401
    import concourse.mybir as mybir
    import concourse.tile as tile
    from concourse import bacc

    f32 = mybir.dt.float32
    bf16 = mybir.dt.bfloat16
    AF = mybir.ActivationFunctionType
    Alu = mybir.AluOpType

    nc = bacc.Bacc("TRN2", target_bir_lowering=False, debug=False,
                   num_devices=N_CORES)

    # ---- external I/O (per-core shards staged by the host) ----
    d_xt = nc.dram_tensor("xt", [NJ, NK, 128, TCOL], bf16, kind="ExternalInput").ap()
    d_xta = nc.dram_tensor("xta", [NK, 128, TCA], bf16, kind="ExternalInput").ap()
    d_wq = nc.dram_tensor("wq", [HIDDEN, 384], bf16, kind="ExternalInput").ap()
    d_kva = nc.dram_tensor("kva", [HIDDEN, KV_RANK + D_ROPE], bf16,
                           kind="ExternalInput").ap()
    d_kvb = nc.dram_tensor("kvb", [KV_RANK, 512], bf16, kind="ExternalInput").ap()
    d_ws1 = nc.dram_tensor("wssq1", [HIDDEN, 128], bf16, kind="ExternalInput").ap()
    d_ws2 = nc.dram_tensor("wssq2", [HIDDEN, 64], bf16, kind="ExternalInput").ap()
    d_ow = nc.dram_tensor("ow", [HPC * D_V, HIDDEN], bf16, kind="ExternalInput").ap()
    d_cos = nc.dram_tensor("cosT", [128, T], f32, kind="ExternalInput").ap()
    d_sin = nc.dram_tensor("sinT", [128, T], f32, kind="ExternalInput").ap()
    d_msw = nc.dram_tensor("mswT", [128, 128], bf16, kind="ExternalInput").ap()
    d_mask = nc.dram_tensor("maskbig", [128, 896], bf16, kind="ExternalInput").ap()
    d_id = nc.dram_tensor("ident", [128, 128], f32, kind="ExternalInput").ap()
    d_out = nc.dram_tensor("out", [NJ, NK, 128, TCOL], bf16,
                           kind="ExternalOutput").ap()

    from contextlib import ExitStack

    with tile.TileContext(nc) as tc, ExitStack() as stk:
        wp = stk.enter_context(tc.tile_pool(name="weights", bufs=1))
        xt_p = stk.enter_context(tc.tile_pool(name="xtp", bufs=2))
        ap_ = stk.enter_context(tc.tile_pool(name="acts", bufs=1))
        sq_p = stk.enter_context(tc.tile_pool(name="sq", bufs=2))
        es_p = stk.enter_context(tc.tile_pool(name="es", bufs=5))
        rp = stk.enter_context(tc.tile_pool(name="rope", bufs=1))
        rd_p = stk.enter_context(tc.tile_pool(name="rdp", bufs=2))
        bc_p = stk.enter_context(tc.tile_pool(name="bcast", bufs=1))
        vt_p = stk.enter_context(tc.tile_pool(name="vtmp", bufs=2))
        o_p = stk.enter_context(tc.tile_pool(name="ocopy", bufs=1))
        dram_p = stk.enter_context(tc.tile_pool(name="dram", bufs=1, space="DRAM"))
        pp = stk.enter_context(tc.tile_pool(name="pp", bufs=2, space="PSUM"))
        ps_p = stk.enter_context(tc.tile_pool(name="ps", bufs=2, space="PSUM"))
        pa_p = stk.enter_context(tc.tile_pool(name="pa", bufs=2, space="PSUM"))
        pd_p = stk.enter_context(tc.tile_pool(name="pd", bufs=2, space="PSUM"))

        # ---- resident tiles ----
        ws1 = wp.tile([128, NK, 128], bf16)
        ws2 = wp.tile([128, NK, 64], bf16)
        wq = wp.tile([128, NK, 384], bf16)

        kvb = wp.tile([128, NL, 512], bf16)
        ow = wp.tile([128, HPC, HIDDEN], bf16)
        cosT = wp.tile([128, T], f32)
        sinT = wp.tile([128, T], f32)
        mswT = wp.tile([128, 128], bf16)
        maskb = wp.tile([128, 896], bf16)
        ident_f32 = wp.tile([128, 128], f32)
        ones = wp.tile([128, 1], bf16)
        # stage-A tensors share the xt stream ring (they are dead after
        # ~30us; ring deps serialize xt col 0/1 loads behind stage A reads)
        kva = xt_p.tile([128, NK, KV_RANK + D_ROPE], bf16, tag="xt",
                        name="kva")
        xta = xt_p.tile([128, NK, KV_RANK + D_ROPE], bf16, tag="xt",
                        name="xta")

        # activations (feature-major / transposed layouts)
        qn = [ap_.tile([128, T], bf16, tag=f"qn{h}", name=f"qn{h}")
              for h in range(HPC)]
        qpe = ap_.tile([128, T], bf16)          # h0 rows 0:64, h1 rows 64:128
        kn = [ap_.tile([128, T], bf16, tag=f"kn{h}", name=f"kn{h}")
              for h in range(HPC)]
        kpe = ap_.tile([128, T], bf16)          # duplicated in both 64-halves
        latT = ap_.tile([128, NL, T], bf16)
        vna = [ap_.tile([128, NS, D_V], bf16, tag=f"v{h}", name=f"v{h}")
               for h in range(HPC)]
        att = [ap_.tile([128, T], bf16, tag=f"att{h}", name=f"att{h}")
               for h in range(HPC)]
        lat_sb = ap_.tile([128, NL, TCA], bf16)
        ch4 = ap_.tile([128, TCA], bf16)
        ssqrow_q = ap_.tile([1, T], bf16)
        ssqa_q = ap_.tile([1, T], bf16)
        ssqkv = ap_.tile([1, T], bf16)
        srow_q = ap_.tile([1, T], f32)          # rsqrt'ed scales (row layout)
        srow_kv = ap_.tile([1, T], f32)
        skvcol_raw = ap_.tile([128, NS], bf16)
        skvcol = ap_.tile([128, NS], f32)

        # ---- initial DMA loads, most-urgent first per queue ----
        # sync queue: xta (stage A input), then the xt stream (in-loop)
        dxta = d_xta.rearrange("k p t -> p k t")
        for g in range(4):
            gs = slice(4 * g, 4 * g + 4)
            nc.sync.dma_start(xta[:, gs, 0:TCA], dxta[:, gs, :])
        # kva split across both HWDGE queues: parallel with xta and the
        # other weights (stage A needs it immediately; SWDGE is ~4x slower)
        dkva = d_kva.rearrange("(k p) c -> p k c", p=128)
        for g in range(4):
            gs = slice(4 * g, 4 * g + 4)
            q = nc.sync if g < 2 else nc.scalar
            q.dma_start(kva[:, gs, :], dkva[:, gs, :])
        dwq = d_wq.rearrange("(k p) c -> p k c", p=128)
        for g in range(4):
            gs = slice(4 * g, 4 * g + 4)
            nc.gpsimd.dma_start(wq[:, gs, :], dwq[:, gs, :])
        nc.gpsimd.memset(ones[:], 1.0)
        # scalar queue: ssq weights, then stage-B weights + small tables
        dws1 = d_ws1.rearrange("(k p) c -> p k c", p=128)
        dws2 = d_ws2.rearrange("(k p) c -> p k c", p=128)
        for g in range(2):
            gs = slice(8 * g, 8 * g + 8)
            nc.scalar.dma_start(ws1[:, gs, :], dws1[:, gs, :])
            nc.scalar.dma_start(ws2[:, gs, :], dws2[:, gs, :])
        nc.scalar.dma_start(kvb[:], d_kvb.rearrange("(k p) c -> p k c", p=128))
        nc.scalar.dma_start(ident_f32[:], d_id[:])
        nc.scalar.dma_start(cosT[:], d_cos[:])
        nc.scalar.dma_start(sinT[:], d_sin[:])
        nc.scalar.dma_start(mswT[:], d_msw[:])
        nc.scalar.dma_start(maskb[:], d_mask[:])
        nc.scalar.dma_start(ow[:], d_ow.rearrange("(h p) c -> p h c", p=128))

        # ---- stage A: kv latent + kv-ssq for this core's 256 tokens ----
        dqa = pd_p.tile([1, TCOL], f32, tag="den", name="dqa")
        for g in range(NL):
            pl = pp.tile([128, TCOL], f32, tag="proj", name=f"latA{g}")
            for k in range(NK):
                nc.tensor.matmul(pl[:, 0:TCA], kva[:, k, 128 * g:128 * g + 128],
                                 xta[:, k, 0:TCA], start=(k == 0),
                                 stop=(k == NK - 1))
            sqa = sq_p.tile([128, TCOL], bf16, tag="sq", name=f"sqa{g}")
            nc.scalar.activation(sqa[:, 0:TCA], pl[:, 0:TCA], AF.Square)
            nc.vector.tensor_copy(lat_sb[:, g, :], pl[:, 0:TCA])
            nc.tensor.matmul(dqa[:, 0:TCA], ones[:, :], sqa[:, 0:TCA],
                             start=(g == 0), stop=(g == NL - 1),
                             skip_group_check=True)
            if g == NL - 1:
                nc.vector.tensor_copy(ch4[64:65, :], dqa[:, 0:TCA])
        plk = pp.tile([128, TCOL], f32, tag="proj", name="latApe")
        for k in range(NK):
            nc.tensor.matmul(plk[0:64, 0:TCA], kva[:, k, KV_RANK:],
                             xta[:, k, 0:TCA], start=(k == 0),
                             stop=(k == NK - 1))
        nc.vector.tensor_copy(ch4[0:64, :], plk[0:64, 0:TCA])

        cc_in = dram_p.tile([5, 128, TCA], bf16)
        cc_out = dram_p.tile([N_CORES, 5, 128, TCA], bf16, addr_space="Shared")
        nc.scalar.dma_start(cc_in[0:NL].rearrange("c p t -> p c t"), lat_sb[:])
        nc.scalar.dma_start(cc_in[NL], ch4[:])
        nc.gpsimd.collective_compute(
            "AllGather", Alu.bypass,
            replica_groups=[list(range(N_CORES))],
            ins=[cc_in.opt()], outs=[cc_out.opt()],
        )
        # ---- rope helper (src: AP of shape [rows, TCOL]) ----
        def rope(dst, src, rows, c):
            e = rp.tile([128, TCOL], f32, tag="re")
            f = rp.tile([128, TCOL], bf16, tag="rf")
            nc.vector.tensor_tensor(e[0:rows, :], src,
                                    cosT[0:rows, c], Alu.mult)
            nc.vector.tensor_tensor(f[0:rows, :], src,
                                    sinT[0:rows, c], Alu.mult)
            pr = ps_p.tile([128, TCOL], f32, tag="score")
            nc.tensor.matmul(pr[0:rows, :], mswT[0:rows, 0:rows], f[0:rows, :],
                             start=True, stop=True)
            nc.vector.tensor_tensor(dst[0:rows, c], e[0:rows, :], pr[0:rows, :],
                                    Alu.add)

        # ---- merged pass: q-ssq + fused q projections (single xt stream) ----
        ar_ins = [dram_p.tile([1, 2 * TCOL], bf16, name=f"ari{x}")
                  for x in range(2)]
        ar_outs = [dram_p.tile([1, 2 * TCOL], bf16, name=f"aro{x}")
                   for x in range(2)]
        for j in range(NJ):
            c = slice(TCOL * j, TCOL * (j + 1))
            xtj = xt_p.tile([128, NK, KV_RANK + D_ROPE], bf16, tag="xt",
                            name=f"xt{j}")
            for g in range(4):
                nc.sync.dma_start(
                    xtj[:, 4 * g:4 * g + 4, 0:TCOL],
                    d_xt[j, 4 * g:4 * g + 4].rearrange("k p t -> p k t"))
            # q-ssq shard
            p0 = pp.tile([128, TCOL], f32, tag="proj")
            for k in range(NK):
                nc.tensor.matmul(p0[:], ws1[:, k, :], xtj[:, k, 0:TCOL],
                                 start=(k == 0), stop=(k == NK - 1))
            s0 = sq_p.tile([128, TCOL], bf16, tag="sq")
            nc.scalar.activation(s0[:], p0[:], AF.Square)
            p1 = pp.tile([128, TCOL], f32, tag="proj")
            for k in range(NK):
                nc.tensor.matmul(p1[0:64, :], ws2[:, k, :], xtj[:, k, 0:TCOL],
                                 start=(k == 0), stop=(k == NK - 1))
            s1 = sq_p.tile([128, TCOL], bf16, tag="sq")
            nc.scalar.activation(s1[0:64, :], p1[0:64, :], AF.Square)
            dq = pd_p.tile([1, TCOL], f32, tag="den")
            nc.tensor.matmul(dq[:], ones[:, :], s0[:], start=True, stop=False)
            nc.tensor.matmul(dq[:], ones[0:64, :], s1[0:64, :],
                             start=False, stop=True)
            nc.vector.tensor_copy(ssqrow_q[0:1, c], dq[:])
            # fused q projections
            for h in range(HPC):
                p = pp.tile([128, TCOL], f32, tag="proj")
                for k in range(NK):
                    nc.tensor.matmul(p[:], wq[:, k, 128 * h:128 * h + 128],
                                     xtj[:, k, 0:TCOL],
                                     start=(k == 0), stop=(k == NK - 1))
                nc.vector.tensor_copy(qn[h][:, c], p[:])
            p = pp.tile([128, TCOL], f32, tag="proj")
            for k in range(NK):
                nc.tensor.matmul(p[:], wq[:, k, 256:384], xtj[:, k, 0:TCOL],
                                 start=(k == 0), stop=(k == NK - 1))
            rope(qpe, p[:, :], 128, c)
            # split q-ssq AllReduce: fire after columns 1 and 3
            if j in (1, 3):
                x = j // 2
                half = slice(TCOL * 2 * x, TCOL * 2 * (x + 1))
                nc.gpsimd.dma_start(ar_ins[x][:], ssqrow_q[0:1, half])
                nc.gpsimd.collective_compute(
                    "AllReduce", Alu.add,
                    replica_groups=[list(range(N_CORES))],
                    ins=[ar_ins[x].opt()], outs=[ar_outs[x].opt()],
                )
                nc.gpsimd.dma_start(ssqa_q[0:1, half], ar_outs[x][:])
                # q scale chain for this half
                nc.vector.tensor_scalar(srow_q[0:1, half], ssqa_q[0:1, half],
                                        1.0 / Q_RANK, EPS, Alu.mult, Alu.add)
                nc.vector.reciprocal_approx_fast(srow_q[0:1, half],
                                                 srow_q[0:1, half])
                nc.scalar.activation(srow_q[0:1, half], srow_q[0:1, half],
                                     AF.Sqrt)

        # unpack the gathered latent.  Emitted AFTER the merged pass so the
        # AllReduce trigger DMAs outrank these in queue priority; latT goes
        # over the sync HWDGE queue (idle once the xt stream is issued).
        for b in range(N_CORES):
            cb = slice(TCA * b, TCA * (b + 1))
            nc.sync.dma_start(latT[:, :, cb],
                              cc_out[b, 0:NL].rearrange("c p t -> p c t"))
            nc.sync.dma_start(kpe[0:64, cb], cc_out[b, NL, 0:64, :])
            nc.gpsimd.dma_start(ssqkv[0:1, cb], cc_out[b, NL, 64:65, :])
            nc.gpsimd.dma_start(
                skvcol_raw[:, 2 * b:2 * b + 2],
                cc_out[b, NL, 64:65, :].rearrange("o (g p) -> (o p) g", p=128))

        # kv scale chains (hoisted, full width)
        nc.vector.tensor_scalar(skvcol[:], skvcol_raw[:], 1.0 / KV_RANK, EPS,
                                Alu.mult, Alu.add)
        nc.vector.reciprocal_approx_fast(skvcol[:], skvcol[:])
        nc.scalar.activation(skvcol[:], skvcol[:], AF.Sqrt)
        nc.vector.tensor_scalar(srow_kv[:], ssqkv[:], 1.0 / KV_RANK, EPS,
                                Alu.mult, Alu.add)
        nc.vector.reciprocal_approx_fast(srow_kv[:], srow_kv[:])
        nc.scalar.activation(srow_kv[:], srow_kv[:], AF.Sqrt)

        # ---- stage B: kn + v from the gathered latent; kpe rope ----
        for j in range(NJ):
            c = slice(TCOL * j, TCOL * (j + 1))
            for h in range(HPC):
                pk = pp.tile([128, TCOL], f32, tag="proj", name=f"pkn{j}_{h}")
                for g in range(NL):
                    nc.tensor.matmul(pk[:], kvb[:, g, 128 * h:128 * h + 128],
                                     latT[:, g, c], start=(g == 0),
                                     stop=(g == NL - 1))
                nc.vector.tensor_copy(kn[h][:, c], pk[:])
            for h in range(HPC):
                pv = pp.tile([128, TCOL], f32, tag="proj", name=f"pvt{j}_{h}")
                for g in range(NL):
                    nc.tensor.matmul(pv[:],
                                     kvb[:, g, 256 + 128 * h:384 + 128 * h],
                                     latT[:, g, c], start=(g == 0),
                                     stop=(g == NL - 1))
                vts = vt_p.tile([128, TCOL], f32, tag="vt")
                nc.vector.tensor_copy(vts[:], pv[:])
                ptr = ps_p.tile([128, TCOL], f32, tag="score",
                                name=f"vtr{j}_{h}")
                for sl in range(4):
                    si = 4 * j + sl
                    nc.tensor.transpose(ptr[:, 128 * sl:128 * sl + 128],
                                        vts[:, 128 * sl:128 * sl + 128],
                                        ident_f32[:])
                    nc.vector.tensor_scalar_mul(
                        vna[h][:, si, :], ptr[:, 128 * sl:128 * sl + 128],
                        skvcol[:, si:si + 1])
            rope(kpe, kpe[0:64, c], 64, c)
            nc.sync.dma_start(kpe[64:128, c], kpe[0:64, c])

        # o_proj helpers: one m-tile at a time so emission can interleave
        # with the NEXT column's attention, filling PE dependency bubbles.
        # Output staged in 4-tile groups (ring of 2) to cap SBUF use.
        ostate = {}

        def emit_oproj_tile(jp, m):
            cp = slice(TCOL * jp, TCOL * (jp + 1))
            if m % 4 == 0:
                ostate['g'] = o_p.tile([128, 4, TCOL], bf16, tag="ot",
                                       name=f"ot{jp}_{m // 4}")
            po = pp.tile([128, TCOL], f32, tag="proj", name=f"po{jp}_{m}")
            for h in range(HPC):
                nc.tensor.matmul(po[:], ow[:, h, 128 * m:128 * m + 128],
                                 att[h][:, cp],
                                 start=(h == 0), stop=(h == HPC - 1))
            nc.vector.tensor_copy(ostate['g'][:, m % 4, :], po[:])
            if m % 4 == 3:
                doj = d_out[jp].rearrange("m p t -> p m t")
                nc.sync.dma_start(doj[:, m - 3:m + 1, :], ostate['g'][:])

        def emit_oproj_col(jp):
            for m in range(NK):
                emit_oproj_tile(jp, m)

        # ---- per-column: late scaling then attention for both heads ----
        for j in range(NJ):
            c = slice(TCOL * j, TCOL * (j + 1))
            sqB = bc_p.tile([128, TCOL], f32, tag="sqB")
            nc.gpsimd.partition_broadcast(sqB[:], srow_q[0:1, c])
            skvB = bc_p.tile([128, TCOL], f32, tag="skvB")
            nc.gpsimd.partition_broadcast(skvB[:], srow_kv[0:1, c])
            for h in range(HPC):
                nc.vector.tensor_tensor(qn[h][:, c], qn[h][:, c], sqB[:],
                                        Alu.mult)
                nc.vector.tensor_tensor(kn[h][:, c], kn[h][:, c], skvB[:],
                                        Alu.mult)
            nc.vector.tensor_tensor(qpe[:, c], qpe[:, c], sqB[:], Alu.mult)

            # attention in S^T[s, t] layout, causal block-skip, software-
            # pipelined emission (den/PV of step i-1 after scores of step i)
            pa2 = [pa_p.tile([128, TCOL], f32, tag="attn", name=f"pa{j}_{h}")
                   for h in range(HPC)]
            pden2 = [pd_p.tile([1, TCOL], f32, tag="den", name=f"pden{j}_{h}")
                     for h in range(HPC)]
            n_s = 4 * (j + 1)
            es_prev = [None, None]
            om = NK
            if j > 0:
                om = 0
                # warm-up filler for the PE while the scale mults run
                while om < 2:
                    emit_oproj_tile(j - 1, om)
                    om += 1

            def emit_pv(i, h):
                nc.tensor.matmul(pden2[h][:], ones[:, :], es_prev[h][:],
                                 start=(i == 0), stop=(i == n_s - 1),
                                 skip_group_check=True)
                nc.tensor.matmul(pa2[h][:], vna[h][:, i, :], es_prev[h][:],
                                 start=(i == 0), stop=(i == n_s - 1),
                                 skip_group_check=True)

            for i in range(n_s):
                for h in range(HPC):
                    st = ps_p.tile([128, TCOL], f32, tag="score")
                    nc.tensor.matmul(st[:], kn[h][:, 128 * i:128 * i + 128],
                                     qn[h][:, c], start=True, stop=False)
                    nc.tensor.matmul(st[:],
                                     kpe[64 * h:64 * h + 64,
                                         128 * i:128 * i + 128],
                                     qpe[64 * h:64 * h + 64, c],
                                     start=False, stop=True)
                    if i > 0:
                        emit_pv(i - 1, h)
                    if i >= 4 * j:
                        ko = i - 4 * j
                        nc.vector.tensor_tensor(
                            st[:], st[:],
                            maskb[:, 384 - 128 * ko:896 - 128 * ko], Alu.add)
                    es = es_p.tile([128, TCOL], bf16, tag="es")
                    nc.scalar.activation(es[:], st[:], AF.Exp)
                    es_prev[h] = es
                # spread previous column's o_proj through this column's
                # attention as ready PE work
                while om < min(NK, (i + 1) * NK // n_s + 2):
                    emit_oproj_tile(j - 1, om)
                    om += 1
            while om < NK:
                emit_oproj_tile(j - 1, om)
                om += 1
            for h in range(HPC):
                emit_pv(n_s - 1, h)
            for h in range(HPC):
                rden = rd_p.tile([1, TCOL], f32, tag="rden")
                nc.vector.reciprocal_approx_fast(rden[:], pden2[h][:])
                rdB = rd_p.tile([128, TCOL], f32, tag="rdB")
                nc.gpsimd.partition_broadcast(rdB[:], rden[:])
                nc.vector.tensor_tensor(att[h][:, c], pa2[h][:], rdB[:],
                                        Alu.mult)

        # o_proj for the last column (earlier columns were emitted inside
        # the following column's attention loop as PE filler work)
        emit_oproj_col(NJ - 1)

    nc.compile()
    return nc


def _host_prep(positions, hidden_states, q_a_w, q_a_ln_w, q_b_w,
               kv_a_w, kv_a_ln_w, kv_b_w, o_w):
    pos = np.asarray(positions, dtype=np.float32)
    hs = np.asarray(hidden_states, dtype=np.float32)
    q_a_w = np.asarray(q_a_w, dtype=np.float32)
    q_b_w = np.asarray(q_b_w, dtype=np.float32) * np.asarray(
        q_a_ln_w, dtype=np.float32)[:, None]
    kv_a_w = np.asarray(kv_a_w, dtype=np.float32)
    kv_b_w = np.asarray(kv_b_w, dtype=np.float32) * np.asarray(
        kv_a_ln_w, dtype=np.float32)[:, None]
    o_w = np.asarray(o_w, dtype=np.float32)

    # fused q weights (softmax scale folded in)
    wq_full = (q_a_w @ q_b_w).reshape(HIDDEN, H, QK_DIM) * SCALE
    kvb = kv_b_w.reshape(KV_RANK, H, D_NOPE + D_V)

    # rope pair permutation: interleaved (0::2, 1::2) -> (x1 block | x2 block)
    qpe_cols = wq_full[:, :, D_NOPE:]
    qpe_perm = np.concatenate([qpe_cols[:, :, 0::2], qpe_cols[:, :, 1::2]],
                              axis=2)  # [HIDDEN, H, 64]
    kva_perm = kv_a_w.copy()
    wkpe = kv_a_w[:, KV_RANK:]
    kva_perm[:, KV_RANK:] = np.concatenate([wkpe[:, 0::2], wkpe[:, 1::2]],
                                           axis=1)

    inv_freq = 1.0 / (ROPE_BASE ** (np.arange(0, D_ROPE, 2,
                                              dtype=np.float32) / D_ROPE))
    freqs = pos[None, :] * inv_freq[:, None]           # [32, T]
    cosT = np.tile(np.cos(freqs).astype(np.float32), (4, 1))   # [128, T]
    sinT = np.tile(np.sin(freqs).astype(np.float32), (4, 1))

    # band-swap-with-sign matrix: o = e + Msw @ f
    msw = np.zeros((128, 128), dtype=np.float32)
    for q in range(2):
        for i in range(32):
            msw[64 * q + i, 64 * q + 32 + i] = -1.0
            msw[64 * q + 32 + i, 64 * q + i] = 1.0
    mswT = np.ascontiguousarray(msw.T).astype(BF16)

    # big causal mask: maskb[s, col] = 0 if col >= s + 384 else NEG
    col = np.arange(896)[None, :]
    s_ = np.arange(128)[:, None]
    maskb = np.where(col >= s_ + 384, 0.0, NEG).astype(BF16)

    hsT = np.ascontiguousarray(hs.T).astype(BF16)      # [HIDDEN, T]
    xt = np.ascontiguousarray(
        hsT.reshape(NK, 128, NJ, TCOL).transpose(2, 0, 1, 3))
    xta_all = hsT.reshape(NK, 128, N_CORES, TCA)

    ident = np.eye(128, dtype=np.float32)

    in_maps = []
    for cidx in range(N_CORES):
        h0 = HPC * cidx
        wq_c = np.concatenate(
            [wq_full[:, h0 + h, :D_NOPE] for h in range(HPC)]
            + [qpe_perm[:, h0 + h, :] for h in range(HPC)], axis=1)
        kvb_c = np.concatenate(
            [kvb[:, h0 + h, :D_NOPE] for h in range(HPC)]
            + [kvb[:, h0 + h, D_NOPE:] for h in range(HPC)], axis=1)
        ws1 = q_a_w[:, 192 * cidx:192 * cidx + 128]
        ws2 = q_a_w[:, 192 * cidx + 128:192 * (cidx + 1)]
        ow_c = o_w[D_V * h0:D_V * (h0 + HPC), :]
        in_maps.append({
            "xt": xt,
            "xta": np.ascontiguousarray(xta_all[:, :, cidx, :]),
            "wq": np.ascontiguousarray(wq_c).astype(BF16),
            "kva": np.ascontiguousarray(kva_perm).astype(BF16),
            "kvb": np.ascontiguousarray(kvb_c).astype(BF16),
            "wssq1": np.ascontiguousarray(ws1).astype(BF16),
            "wssq2": np.ascontiguousarray(ws2).astype(BF16),
            "ow": np.ascontiguousarray(ow_c).astype(BF16),
            "cosT": cosT,
            "sinT": sinT,
            "mswT": mswT,
            "maskbig": maskb,
            "ident": ident,
        })
    return in_maps


def kernel(**inputs):
    from concourse.bass_utils import run_bass_kernel_spmd

    if "nc" not in _CACHE:
        _CACHE["nc"] = _build_program()
    nc = _CACHE["nc"]

    in_maps = _host_prep(**inputs)
    trace = bool(int(os.environ.get("BASSK_TRACE", "0")))
    tmpdir = os.environ.get("BASSK_TMPDIR") or None
    if tmpdir:
        os.makedirs(tmpdir, exist_ok=True)
    res = run_bass_kernel_spmd(nc, in_maps, core_ids=list(range(N_CORES)),
                               trace=trace, tmpdir=tmpdir)
    _CACHE["last_exec_time_ns"] = res.exec_time_ns
    _CACHE["last_results"] = res.results
    outT = np.zeros((NJ, NK, 128, TCOL), dtype=np.float32)
    for r in res.results:
        outT += np.asarray(r["out"], dtype=np.float32)
    outT = outT.transpose(1, 2, 0, 3).reshape(HIDDEN, T)
    return np.ascontiguousarray(outT.T)


# revision 15
# speedup vs baseline: 1.0101x; 1.0101x over previous
"""DeepseekV2 MLA attention on 8 TRN2 NeuronCores (Bass/Tile), v2.

Strategy (tensor-parallel over heads, 2 heads/core), changes vs v1:
  - KV path is two-stage: stage A computes the full 576-dim kv_a latent for
    this core's own 256 tokens (dedicated xta input slice), one AllGather
    (~21us) shares the latent + per-token kv ssq across cores, stage B
    (latent @ kv_b, 512-contraction) replaces the fused wk/wv/wkpe
    projections (2048-contraction): ~120k fewer PE columns per core.
  - V comes out of stage B feature-major; PE-transposed into natural [s, dv]
    tiles with the RMSNorm scale fused into the per-partition copy.
  - The q-ssq pass shares the single xt stream with the wq projections
    (xt streamed once, not twice); its AllReduce is split in two so early
    attention columns unblock sooner.
  - All reciprocals via reciprocal_approx_fast (5x; the [1,512]
    single-partition reciprocals were 3.3us each on DVE).
  - RoPE pairs host-permuted to block layout; softmax scale folded into Wq;
    RMSNorm ln weights folded into the following projections (exact).
  - Attention in S^T[s,t] layout: scores via PE (k stationary), exp on ACT
    (no max subtraction; |scores| ~ O(1)), causal masking via additive mask
    on diagonal tiles, denominator via ones-vector matmul, PV with natural-
    layout V stationary.  o_proj row-parallel, host sums 8 partials.
"""

import os
import sys

import numpy as np

for _p in ("/opt/trn_rl_repo",):
    if _p not in sys.path and os.path.isdir(_p):
        sys.path.insert(0, _p)

import ml_dtypes  # noqa: E402

BF16 = ml_dtypes.bfloat16

H = 16
D_NOPE = 128
D_ROPE = 64
D_V = 128
KV_RANK = 512
Q_RANK = 1536
HIDDEN = 2048
T = 2048
EPS = 1e-6
QK_DIM = D_NOPE + D_ROPE
SCALE = QK_DIM ** -0.5
ROPE_BASE = 10000.0

N_CORES = 8
HPC = H // N_CORES          # heads per core = 2
TCOL = 512                  # moving-operand width
TCA = 256                   # stage-A token slice per core
NJ = T // TCOL              # 4 t-column blocks
NK = HIDDEN // 128          # 16 contraction chunks
NL = KV_RANK // 128         # 4 latent chunks
NS = T // 128               # 16 key tiles
NEG = -1.0e4                # causal mask additive value

_CACHE = {}


def _build_program():
    import concourse.bass as bass  # noqa: F401
    import concourse.mybir as mybir
    import concourse.tile as tile
    from concourse import bacc

    f32 = mybir.dt.float32
    bf16 = mybir.dt.bfloat16
    AF = mybir.ActivationFunctionType
    Alu = mybir.AluOpType

    nc = bacc.Bacc("TRN2", target_bir_lowering=False, debug=False,
                   num_devices=N_CORES)

    # ---- external I/O (per-core shards staged by the host) ----
    d_xt = nc.dram_tensor("xt", [NJ, NK, 128, TCOL], bf16, kind="ExternalInput").ap()
    d_xta = nc.dram_tensor("xta", [NK, 128, TCA], bf16, kind="ExternalInput").ap()
    d_wq = nc.dram_tensor("wq", [HIDDEN, 384], bf16, kind="ExternalInput").ap()
    d_kva = nc.dram_tensor("kva", [HIDDEN, KV_RANK + D_ROPE], bf16,
                           kind="ExternalInput").ap()
    d_kvb = nc.dram_tensor("kvb", [KV_RANK, 512], bf16, kind="ExternalInput").ap()
    d_ws1 = nc.dram_tensor("wssq1", [HIDDEN, 128], bf16, kind="ExternalInput").ap()
    d_ws2 = nc.dram_tensor("wssq2", [HIDDEN, 64], bf16, kind="ExternalInput").ap()
    d_ow = nc.dram_tensor("ow", [HPC * D_V, HIDDEN], bf16, kind="ExternalInput").ap()
    d_cos = nc.dram_tensor("cosT", [128, T], f32, kind="ExternalInput").ap()
    d_sin = nc.dram_tensor("sinT", [128, T], f32, kind="ExternalInput").ap()
    d_msw = nc.dram_tensor("mswT", [128, 128], bf16, kind="ExternalInput").ap()
    d_mask = nc.dram_tensor("maskbig", [128, 896], bf16, kind="ExternalInput").ap()
    d_id = nc.dram_tensor("ident", [128, 128], f32, kind="ExternalInput").ap()
    d_out = nc.dram_tensor("out", [NJ, NK, 128, TCOL], bf16,
                           kind="ExternalOutput").ap()

    from contextlib import ExitStack

    with tile.TileContext(nc) as tc, ExitStack() as stk:
        wp = stk.enter_context(tc.tile_pool(name="weights", bufs=1))
        xt_p = stk.enter_context(tc.tile_pool(name="xtp", bufs=2))
        ap_ = stk.enter_context(tc.tile_pool(name="acts", bufs=1))
        sq_p = stk.enter_context(tc.tile_pool(name="sq", bufs=2))
        es_p = stk.enter_context(tc.tile_pool(name="es", bufs=5))
        rp = stk.enter_context(tc.tile_pool(name="rope", bufs=1))
        rd_p = stk.enter_context(tc.tile_pool(name="rdp", bufs=2))
        bc_p = stk.enter_context(tc.tile_pool(name="bcast", bufs=1))
        vt_p = stk.enter_context(tc.tile_pool(name="vtmp", bufs=2))
        o_p = stk.enter_context(tc.tile_pool(name="ocopy", bufs=1))
        dram_p = stk.enter_context(tc.tile_pool(name="dram", bufs=1, space="DRAM"))
        pp = stk.enter_context(tc.tile_pool(name="pp", bufs=2, space="PSUM"))
        ps_p = stk.enter_context(tc.tile_pool(name="ps", bufs=2, space="PSUM"))
        pa_p = stk.enter_context(tc.tile_pool(name="pa", bufs=2, space="PSUM"))
        pd_p = stk.enter_context(tc.tile_pool(name="pd", bufs=2, space="PSUM"))

        # ---- resident tiles ----
        ws1 = wp.tile([128, NK, 128], bf16)
        ws2 = wp.tile([128, NK, 64], bf16)
        wq = wp.tile([128, NK, 384], bf16)

        kvb = wp.tile([128, NL, 512], bf16)
        ow = wp.tile([128, HPC, HIDDEN], bf16)
        cosT = wp.tile([128, T], f32)
        sinT = wp.tile([128, T], f32)
        mswT = wp.tile([128, 128], bf16)
        maskb = wp.tile([128, 896], bf16)
        ident_f32 = wp.tile([128, 128], f32)
        ones = wp.tile([128, 1], bf16)
        # stage-A tensors share the xt stream ring (they are dead after
        # ~30us; ring deps serialize xt col 0/1 loads behind stage A reads)
        kva = xt_p.tile([128, NK, KV_RANK + D_ROPE], bf16, tag="xt",
                        name="kva")
        xta = xt_p.tile([128, NK, KV_RANK + D_ROPE], bf16, tag="xt",
                        name="xta")

        # activations (feature-major / transposed layouts)
        qn = [ap_.tile([128, T], bf16, tag=f"qn{h}", name=f"qn{h}")
              for h in range(HPC)]
        qpe = ap_.tile([128, T], bf16)          # h0 rows 0:64, h1 rows 64:128
        kn = [ap_.tile([128, T], bf16, tag=f"kn{h}", name=f"kn{h}")
              for h in range(HPC)]
        kpe = ap_.tile([128, T], bf16)          # duplicated in both 64-halves
        latT = ap_.tile([128, NL, T], bf16)
        vna = [ap_.tile([128, NS, D_V], bf16, tag=f"v{h}", name=f"v{h}")
               for h in range(HPC)]
        att = [ap_.tile([128, T], bf16, tag=f"att{h}", name=f"att{h}")
               for h in range(HPC)]
        lat_sb = ap_.tile([128, NL, TCA], bf16)
        ch4 = ap_.tile([128, TCA], bf16)
        ssqrow_q = ap_.tile([1, T], bf16)
        ssqa_q = ap_.tile([1, T], bf16)
        ssqkv = ap_.tile([1, T], bf16)
        srow_q = ap_.tile([1, T], f32)          # rsqrt'ed scales (row layout)
        srow_kv = ap_.tile([1, T], f32)
        skvcol_raw = ap_.tile([128, NS], bf16)
        skvcol = ap_.tile([128, NS], f32)

        # ---- initial DMA loads, most-urgent first per queue ----
        # stage A inputs lead both HWDGE queues, k-interleaved so the first
        # matmuls can start after ~1MB instead of the whole first wave
        dxta = d_xta.rearrange("k p t -> p k t")
        dkva = d_kva.rearrange("(k p) c -> p k c", p=128)
        for g in range(4):
            gs = slice(4 * g, 4 * g + 4)
            q = nc.sync if g < 2 else nc.scalar
            q.dma_start(kva[:, gs, :], dkva[:, gs, :])
            nc.sync.dma_start(xta[:, gs, 0:TCA], dxta[:, gs, :])
        dwq = d_wq.rearrange("(k p) c -> p k c", p=128)
        for g in range(4):
            gs = slice(4 * g, 4 * g + 4)
            nc.gpsimd.dma_start(wq[:, gs, :], dwq[:, gs, :])
        nc.gpsimd.memset(ones[:], 1.0)
        # scalar queue: ssq weights, then stage-B weights + small tables
        dws1 = d_ws1.rearrange("(k p) c -> p k c", p=128)
        dws2 = d_ws2.rearrange("(k p) c -> p k c", p=128)
        for g in range(2):
            gs = slice(8 * g, 8 * g + 8)
            nc.scalar.dma_start(ws1[:, gs, :], dws1[:, gs, :])
            nc.scalar.dma_start(ws2[:, gs, :], dws2[:, gs, :])
        nc.scalar.dma_start(cosT[:], d_cos[:])
        nc.scalar.dma_start(sinT[:], d_sin[:])
        nc.scalar.dma_start(mswT[:], d_msw[:])

        # ---- stage A: kv latent + kv-ssq for this core's 256 tokens ----
        dqa = pd_p.tile([1, TCOL], f32, tag="den", name="dqa")
        for g in range(NL):
            pl = pp.tile([128, TCOL], f32, tag="proj", name=f"latA{g}")
            for k in range(NK):
                nc.tensor.matmul(pl[:, 0:TCA], kva[:, k, 128 * g:128 * g + 128],
                                 xta[:, k, 0:TCA], start=(k == 0),
                                 stop=(k == NK - 1))
            sqa = sq_p.tile([128, TCOL], bf16, tag="sq", name=f"sqa{g}")
            nc.scalar.activation(sqa[:, 0:TCA], pl[:, 0:TCA], AF.Square)
            nc.vector.tensor_copy(lat_sb[:, g, :], pl[:, 0:TCA])
            nc.tensor.matmul(dqa[:, 0:TCA], ones[:, :], sqa[:, 0:TCA],
                             start=(g == 0), stop=(g == NL - 1),
                             skip_group_check=True)
            if g == NL - 1:
                nc.vector.tensor_copy(ch4[64:65, :], dqa[:, 0:TCA])
        plk = pp.tile([128, TCOL], f32, tag="proj", name="latApe")
        for k in range(NK):
            nc.tensor.matmul(plk[0:64, 0:TCA], kva[:, k, KV_RANK:],
                             xta[:, k, 0:TCA], start=(k == 0),
                             stop=(k == NK - 1))
        nc.vector.tensor_copy(ch4[0:64, :], plk[0:64, 0:TCA])

        cc_in = dram_p.tile([5, 128, TCA], bf16)
        cc_out = dram_p.tile([N_CORES, 5, 128, TCA], bf16, addr_space="Shared")
        nc.scalar.dma_start(cc_in[0:NL].rearrange("c p t -> p c t"), lat_sb[:])
        nc.scalar.dma_start(cc_in[NL], ch4[:])
        nc.gpsimd.collective_compute(
            "AllGather", Alu.bypass,
            replica_groups=[list(range(N_CORES))],
            ins=[cc_in.opt()], outs=[cc_out.opt()],
        )
        # ---- rope helper (src: AP of shape [rows, TCOL]) ----
        def rope(dst, src, rows, c):
            e = rp.tile([128, TCOL], f32, tag="re")
            f = rp.tile([128, TCOL], bf16, tag="rf")
            nc.vector.tensor_tensor(e[0:rows, :], src,
                                    cosT[0:rows, c], Alu.mult)
            nc.vector.tensor_tensor(f[0:rows, :], src,
                                    sinT[0:rows, c], Alu.mult)
            pr = ps_p.tile([128, TCOL], f32, tag="score")
            nc.tensor.matmul(pr[0:rows, :], mswT[0:rows, 0:rows], f[0:rows, :],
                             start=True, stop=True)
            nc.vector.tensor_tensor(dst[0:rows, c], e[0:rows, :], pr[0:rows, :],
                                    Alu.add)

        # ---- merged pass: q-ssq + fused q projections (single xt stream) ----
        ar_ins = [dram_p.tile([1, 2 * TCOL], bf16, name=f"ari{x}")
                  for x in range(2)]
        ar_outs = [dram_p.tile([1, 2 * TCOL], bf16, name=f"aro{x}")
                   for x in range(2)]
        for j in range(NJ):
            c = slice(TCOL * j, TCOL * (j + 1))
            xtj = xt_p.tile([128, NK, KV_RANK + D_ROPE], bf16, tag="xt",
                            name=f"xt{j}")
            for g in range(4):
                nc.sync.dma_start(
                    xtj[:, 4 * g:4 * g + 4, 0:TCOL],
                    d_xt[j, 4 * g:4 * g + 4].rearrange("k p t -> p k t"))
            # q-ssq shard
            p0 = pp.tile([128, TCOL], f32, tag="proj")
            for k in range(NK):
                nc.tensor.matmul(p0[:], ws1[:, k, :], xtj[:, k, 0:TCOL],
                                 start=(k == 0), stop=(k == NK - 1))
            s0 = sq_p.tile([128, TCOL], bf16, tag="sq")
            nc.scalar.activation(s0[:], p0[:], AF.Square)
            p1 = pp.tile([128, TCOL], f32, tag="proj")
            for k in range(NK):
                nc.tensor.matmul(p1[0:64, :], ws2[:, k, :], xtj[:, k, 0:TCOL],
                                 start=(k == 0), stop=(k == NK - 1))
            s1 = sq_p.tile([128, TCOL], bf16, tag="sq")
            nc.scalar.activation(s1[0:64, :], p1[0:64, :], AF.Square)
            dq = pd_p.tile([1, TCOL], f32, tag="den")
            nc.tensor.matmul(dq[:], ones[:, :], s0[:], start=True, stop=False)
            nc.tensor.matmul(dq[:], ones[0:64, :], s1[0:64, :],
                             start=False, stop=True)
            nc.vector.tensor_copy(ssqrow_q[0:1, c], dq[:])
            # fused q projections
            for h in range(HPC):
                p = pp.tile([128, TCOL], f32, tag="proj")
                for k in range(NK):
                    nc.tensor.matmul(p[:], wq[:, k, 128 * h:128 * h + 128],
                                     xtj[:, k, 0:TCOL],
                                     start=(k == 0), stop=(k == NK - 1))
                nc.vector.tensor_copy(qn[h][:, c], p[:])
            p = pp.tile([128, TCOL], f32, tag="proj")
            for k in range(NK):
                nc.tensor.matmul(p[:], wq[:, k, 256:384], xtj[:, k, 0:TCOL],
                                 start=(k == 0), stop=(k == NK - 1))
            rope(qpe, p[:, :], 128, c)
            # split q-ssq AllReduce: fire after columns 1 and 3
            if j in (1, 3):
                x = j // 2
                half = slice(TCOL * 2 * x, TCOL * 2 * (x + 1))
                nc.gpsimd.dma_start(ar_ins[x][:], ssqrow_q[0:1, half])
                nc.gpsimd.collective_compute(
                    "AllReduce", Alu.add,
                    replica_groups=[list(range(N_CORES))],
                    ins=[ar_ins[x].opt()], outs=[ar_outs[x].opt()],
                )
                nc.gpsimd.dma_start(ssqa_q[0:1, half], ar_outs[x][:])
                # q scale chain for this half
                nc.vector.tensor_scalar(srow_q[0:1, half], ssqa_q[0:1, half],
                                        1.0 / Q_RANK, EPS, Alu.mult, Alu.add)
                nc.vector.reciprocal_approx_fast(srow_q[0:1, half],
                                                 srow_q[0:1, half])
                nc.scalar.activation(srow_q[0:1, half], srow_q[0:1, half],
                                     AF.Sqrt)

        # deferred weights: not needed before ~85us, keep early HBM free
        nc.scalar.dma_start(kvb[:], d_kvb.rearrange("(k p) c -> p k c", p=128))
        nc.scalar.dma_start(ident_f32[:], d_id[:])
        nc.scalar.dma_start(maskb[:], d_mask[:])
        nc.scalar.dma_start(ow[:], d_ow.rearrange("(h p) c -> p h c", p=128))

        # unpack the gathered latent.  Emitted AFTER the merged pass so the
        # AllReduce trigger DMAs outrank these in queue priority; latT goes
        # over the sync HWDGE queue (idle once the xt stream is issued).
        for b in range(N_CORES):
            cb = slice(TCA * b, TCA * (b + 1))
            nc.sync.dma_start(latT[:, :, cb],
                              cc_out[b, 0:NL].rearrange("c p t -> p c t"))
            nc.sync.dma_start(kpe[0:64, cb], cc_out[b, NL, 0:64, :])
            nc.gpsimd.dma_start(ssqkv[0:1, cb], cc_out[b, NL, 64:65, :])
            nc.gpsimd.dma_start(
                skvcol_raw[:, 2 * b:2 * b + 2],
                cc_out[b, NL, 64:65, :].rearrange("o (g p) -> (o p) g", p=128))

        # kv scale chains (hoisted, full width)
        nc.vector.tensor_scalar(skvcol[:], skvcol_raw[:], 1.0 / KV_RANK, EPS,
                                Alu.mult, Alu.add)
        nc.vector.reciprocal_approx_fast(skvcol[:], skvcol[:])
        nc.scalar.activation(skvcol[:], skvcol[:], AF.Sqrt)
        nc.vector.tensor_scalar(srow_kv[:], ssqkv[:], 1.0 / KV_RANK, EPS,
                                Alu.mult, Alu.add)
        nc.vector.reciprocal_approx_fast(srow_kv[:], srow_kv[:])
        nc.scalar.activation(srow_kv[:], srow_kv[:], AF.Sqrt)

        # ---- stage B: kn + v from the gathered latent; kpe rope ----
        for j in range(NJ):
            c = slice(TCOL * j, TCOL * (j + 1))
            for h in range(HPC):
                pk = pp.tile([128, TCOL], f32, tag="proj", name=f"pkn{j}_{h}")
                for g in range(NL):
                    nc.tensor.matmul(pk[:], kvb[:, g, 128 * h:128 * h + 128],
                                     latT[:, g, c], start=(g == 0),
                                     stop=(g == NL - 1))
                nc.vector.tensor_copy(kn[h][:, c], pk[:])
            for h in range(HPC):
                pv = pp.tile([128, TCOL], f32, tag="proj", name=f"pvt{j}_{h}")
                for g in range(NL):
                    nc.tensor.matmul(pv[:],
                                     kvb[:, g, 256 + 128 * h:384 + 128 * h],
                                     latT[:, g, c], start=(g == 0),
                                     stop=(g == NL - 1))
                vts = vt_p.tile([128, TCOL], f32, tag="vt")
                nc.vector.tensor_copy(vts[:], pv[:])
                ptr = ps_p.tile([128, TCOL], f32, tag="score",
                                name=f"vtr{j}_{h}")
                for sl in range(4):
                    si = 4 * j + sl
                    nc.tensor.transpose(ptr[:, 128 * sl:128 * sl + 128],
                                        vts[:, 128 * sl:128 * sl + 128],
                                        ident_f32[:])
                    nc.vector.tensor_scalar_mul(
                        vna[h][:, si, :], ptr[:, 128 * sl:128 * sl + 128],
                        skvcol[:, si:si + 1])
            rope(kpe, kpe[0:64, c], 64, c)
            nc.sync.dma_start(kpe[64:128, c], kpe[0:64, c])

        # o_proj helpers: one m-tile at a time so emission can interleave
        # with the NEXT column's attention, filling PE dependency bubbles.
        # Output staged in 4-tile groups (ring of 2) to cap SBUF use.
        ostate = {}

        def emit_oproj_tile(jp, m):
            cp = slice(TCOL * jp, TCOL * (jp + 1))
            if m % 4 == 0:
                ostate['g'] = o_p.tile([128, 4, TCOL], bf16, tag="ot",
                                       name=f"ot{jp}_{m // 4}")
            po = pp.tile([128, TCOL], f32, tag="proj", name=f"po{jp}_{m}")
            for h in range(HPC):
                nc.tensor.matmul(po[:], ow[:, h, 128 * m:128 * m + 128],
                                 att[h][:, cp],
                                 start=(h == 0), stop=(h == HPC - 1))
            eng = nc.vector if m % 2 == 0 else nc.scalar
            if eng is nc.vector:
                eng.tensor_copy(ostate['g'][:, m % 4, :], po[:])
            else:
                nc.scalar.copy(ostate['g'][:, m % 4, :], po[:])
            if m % 4 == 3:
                doj = d_out[jp].rearrange("m p t -> p m t")
                nc.sync.dma_start(doj[:, m - 3:m + 1, :], ostate['g'][:])

        def emit_oproj_col(jp):
            for m in range(NK):
                emit_oproj_tile(jp, m)

        # ---- per-column: late scaling then attention for both heads ----
        for j in range(NJ):
            c = slice(TCOL * j, TCOL * (j + 1))
            sqB = bc_p.tile([128, TCOL], f32, tag="sqB")
            nc.gpsimd.partition_broadcast(sqB[:], srow_q[0:1, c])
            skvB = bc_p.tile([128, TCOL], f32, tag="skvB")
            nc.gpsimd.partition_broadcast(skvB[:], srow_kv[0:1, c])
            for h in range(HPC):
                nc.vector.tensor_tensor(qn[h][:, c], qn[h][:, c], sqB[:],
                                        Alu.mult)
                nc.vector.tensor_tensor(kn[h][:, c], kn[h][:, c], skvB[:],
                                        Alu.mult)
            nc.vector.tensor_tensor(qpe[:, c], qpe[:, c], sqB[:], Alu.mult)

            # attention in S^T[s, t] layout, causal block-skip, software-
            # pipelined emission (den/PV of step i-1 after scores of step i)
            pa2 = [pa_p.tile([128, TCOL], f32, tag="attn", name=f"pa{j}_{h}")
                   for h in range(HPC)]
            pden2 = [pd_p.tile([1, TCOL], f32, tag="den", name=f"pden{j}_{h}")
                     for h in range(HPC)]
            n_s = 4 * (j + 1)
            es_prev = [None, None]
            om = NK
            if j > 0:
                om = 0
                # warm-up filler for the PE while the scale mults run
                while om < 2:
                    emit_oproj_tile(j - 1, om)
                    om += 1

            def emit_pv(i, h):
                nc.tensor.matmul(pden2[h][:], ones[:, :], es_prev[h][:],
                                 start=(i == 0), stop=(i == n_s - 1),
                                 skip_group_check=True)
                nc.tensor.matmul(pa2[h][:], vna[h][:, i, :], es_prev[h][:],
                                 start=(i == 0), stop=(i == n_s - 1),
                                 skip_group_check=True)

            for i in range(n_s):
                for h in range(HPC):
                    st = ps_p.tile([128, TCOL], f32, tag="score")
                    nc.tensor.matmul(st[:], kn[h][:, 128 * i:128 * i + 128],
                                     qn[h][:, c], start=True, stop=False)
                    nc.tensor.matmul(st[:],
                                     kpe[64 * h:64 * h + 64,
                                         128 * i:128 * i + 128],
                                     qpe[64 * h:64 * h + 64, c],
                                     start=False, stop=True)
                    if i > 0:
                        emit_pv(i - 1, h)
                    if i >= 4 * j:
                        ko = i - 4 * j
                        nc.vector.tensor_tensor(
                            st[:], st[:],
                            maskb[:, 384 - 128 * ko:896 - 128 * ko], Alu.add)
                    es = es_p.tile([128, TCOL], bf16, tag="es")
                    nc.scalar.activation(es[:], st[:], AF.Exp)
                    es_prev[h] = es
                # spread previous column's o_proj through this column's
                # attention as ready PE work
                while om < min(NK, (i + 1) * NK // n_s + 2):
                    emit_oproj_tile(j - 1, om)
                    om += 1
            while om < NK:
                emit_oproj_tile(j - 1, om)
                om += 1
            for h in range(HPC):
                emit_pv(n_s - 1, h)
            for h in range(HPC):
                rden = rd_p.tile([1, TCOL], f32, tag="rden")
                nc.vector.reciprocal_approx_fast(rden[:], pden2[h][:])
                rdB = rd_p.tile([128, TCOL], f32, tag="rdB")
                nc.gpsimd.partition_broadcast(rdB[:], rden[:])
                nc.vector.tensor_tensor(att[h][:, c], pa2[h][:], rdB[:],
                                        Alu.mult)

        # o_proj for the last column (earlier columns were emitted inside
        # the following column's attention loop as PE filler work)
        emit_oproj_col(NJ - 1)

    nc.compile()
    return nc


def _host_prep(positions, hidden_states, q_a_w, q_a_ln_w, q_b_w,
               kv_a_w, kv_a_ln_w, kv_b_w, o_w):
    pos = np.asarray(positions, dtype=np.float32)
    hs = np.asarray(hidden_states, dtype=np.float32)
    q_a_w = np.asarray(q_a_w, dtype=np.float32)
    q_b_w = np.asarray(q_b_w, dtype=np.float32) * np.asarray(
        q_a_ln_w, dtype=np.float32)[:, None]
    kv_a_w = np.asarray(kv_a_w, dtype=np.float32)
    kv_b_w = np.asarray(kv_b_w, dtype=np.float32) * np.asarray(
        kv_a_ln_w, dtype=np.float32)[:, None]
    o_w = np.asarray(o_w, dtype=np.float32)

    # fused q weights (softmax scale folded in)
    wq_full = (q_a_w @ q_b_w).reshape(HIDDEN, H, QK_DIM) * SCALE
    kvb = kv_b_w.reshape(KV_RANK, H, D_NOPE + D_V)

    # rope pair permutation: interleaved (0::2, 1::2) -> (x1 block | x2 block)
    qpe_cols = wq_full[:, :, D_NOPE:]
    qpe_perm = np.concatenate([qpe_cols[:, :, 0::2], qpe_cols[:, :, 1::2]],
                              axis=2)  # [HIDDEN, H, 64]
    kva_perm = kv_a_w.copy()
    wkpe = kv_a_w[:, KV_RANK:]
    kva_perm[:, KV_RANK:] = np.concatenate([wkpe[:, 0::2], wkpe[:, 1::2]],
                                           axis=1)

    inv_freq = 1.0 / (ROPE_BASE ** (np.arange(0, D_ROPE, 2,
                                              dtype=np.float32) / D_ROPE))
    freqs = pos[None, :] * inv_freq[:, None]           # [32, T]
    cosT = np.tile(np.cos(freqs).astype(np.float32), (4, 1))   # [128, T]
    sinT = np.tile(np.sin(freqs).astype(np.float32), (4, 1))

    # band-swap-with-sign matrix: o = e + Msw @ f
    msw = np.zeros((128, 128), dtype=np.float32)
    for q in range(2):
        for i in range(32):
            msw[64 * q + i, 64 * q + 32 + i] = -1.0
            msw[64 * q + 32 + i, 64 * q + i] = 1.0
    mswT = np.ascontiguousarray(msw.T).astype(BF16)

    # big causal mask: maskb[s, col] = 0 if col >= s + 384 else NEG
    col = np.arange(896)[None, :]
    s_ = np.arange(128)[:, None]
    maskb = np.where(col >= s_ + 384, 0.0, NEG).astype(BF16)

    hsT = np.ascontiguousarray(hs.T).astype(BF16)      # [HIDDEN, T]
    xt = np.ascontiguousarray(
        hsT.reshape(NK, 128, NJ, TCOL).transpose(2, 0, 1, 3))
    xta_all = hsT.reshape(NK, 128, N_CORES, TCA)

    ident = np.eye(128, dtype=np.float32)

    in_maps = []
    for cidx in range(N_CORES):
        h0 = HPC * cidx
        wq_c = np.concatenate(
            [wq_full[:, h0 + h, :D_NOPE] for h in range(HPC)]
            + [qpe_perm[:, h0 + h, :] for h in range(HPC)], axis=1)
        kvb_c = np.concatenate(
            [kvb[:, h0 + h, :D_NOPE] for h in range(HPC)]
            + [kvb[:, h0 + h, D_NOPE:] for h in range(HPC)], axis=1)
        ws1 = q_a_w[:, 192 * cidx:192 * cidx + 128]
        ws2 = q_a_w[:, 192 * cidx + 128:192 * (cidx + 1)]
        ow_c = o_w[D_V * h0:D_V * (h0 + HPC), :]
        in_maps.append({
            "xt": xt,
            "xta": np.ascontiguousarray(xta_all[:, :, cidx, :]),
            "wq": np.ascontiguousarray(wq_c).astype(BF16),
            "kva": np.ascontiguousarray(kva_perm).astype(BF16),
            "kvb": np.ascontiguousarray(kvb_c).astype(BF16),
            "wssq1": np.ascontiguousarray(ws1).astype(BF16),
            "wssq2": np.ascontiguousarray(ws2).astype(BF16),
            "ow": np.ascontiguousarray(ow_c).astype(BF16),
            "cosT": cosT,
            "sinT": sinT,
            "mswT": mswT,
            "maskbig": maskb,
            "ident": ident,
        })
    return in_maps


def kernel(**inputs):
    from concourse.bass_utils import run_bass_kernel_spmd

    if "nc" not in _CACHE:
        _CACHE["nc"] = _build_program()
    nc = _CACHE["nc"]

    in_maps = _host_prep(**inputs)
    trace = bool(int(os.environ.get("BASSK_TRACE", "0")))
    tmpdir = os.environ.get("BASSK_TMPDIR") or None
    if tmpdir:
        os.makedirs(tmpdir, exist_ok=True)
    res = run_bass_kernel_spmd(nc, in_maps, core_ids=list(range(N_CORES)),
                               trace=trace, tmpdir=tmpdir)
    _CACHE["last_exec_time_ns"] = res.exec_time_ns
    _CACHE["last_results"] = res.results
    outT = np.zeros((NJ, NK, 128, TCOL), dtype=np.float32)
    for r in res.results:
        outT += np.asarray(r["out"], dtype=np.float32)
    outT = outT.transpose(1, 2, 0, 3).reshape(HIDDEN, T)
    return np.ascontiguousarray(outT.T)


# revision 16
# speedup vs baseline: 1.0384x; 1.0280x over previous
"""DeepseekV2 MLA attention on 8 TRN2 NeuronCores (Bass/Tile), v2.

Strategy (tensor-parallel over heads, 2 heads/core), changes vs v1:
  - KV path is two-stage: stage A computes the full 576-dim kv_a latent for
    this core's own 256 tokens (dedicated xta input slice), one AllGather
    (~21us) shares the latent + per-token kv ssq across cores, stage B
    (latent @ kv_b, 512-contraction) replaces the fused wk/wv/wkpe
    projections (2048-contraction): ~120k fewer PE columns per core.
  - V comes out of stage B feature-major; PE-transposed into natural [s, dv]
    tiles with the RMSNorm scale fused into the per-partition copy.
  - The q-ssq pass shares the single xt stream with the wq projections
    (xt streamed once, not twice); its AllReduce is split in two so early
    attention columns unblock sooner.
  - All reciprocals via reciprocal_approx_fast (5x; the [1,512]
    single-partition reciprocals were 3.3us each on DVE).
  - RoPE pairs host-permuted to block layout; softmax scale folded into Wq;
    RMSNorm ln weights folded into the following projections (exact).
  - Attention in S^T[s,t] layout: scores via PE (k stationary), exp on ACT
    (no max subtraction; |scores| ~ O(1)), causal masking via additive mask
    on diagonal tiles, denominator via ones-vector matmul, PV with natural-
    layout V stationary.  o_proj row-parallel, host sums 8 partials.
"""

import os
import sys

import numpy as np

for _p in ("/opt/trn_rl_repo",):
    if _p not in sys.path and os.path.isdir(_p):
        sys.path.insert(0, _p)

import ml_dtypes  # noqa: E402

BF16 = ml_dtypes.bfloat16

H = 16
D_NOPE = 128
D_ROPE = 64
D_V = 128
KV_RANK = 512
Q_RANK = 1536
HIDDEN = 2048
T = 2048
EPS = 1e-6
QK_DIM = D_NOPE + D_ROPE
SCALE = QK_DIM ** -0.5
ROPE_BASE = 10000.0

N_CORES = 8
HPC = H // N_CORES          # heads per core = 2
TCOL = 512                  # moving-operand width
TCA = 256                   # stage-A token slice per core
NJ = T // TCOL              # 4 t-column blocks
NK = HIDDEN // 128          # 16 contraction chunks
NL = KV_RANK // 128         # 4 latent chunks
NS = T // 128               # 16 key tiles
NEG = -1.0e4                # causal mask additive value

_CACHE = {}


def _build_program():
    import concourse.bass as bass  # noqa: F401
    import concourse.mybir as mybir
    import concourse.tile as tile
    from concourse import bacc

    f32 = mybir.dt.float32
    bf16 = mybir.dt.bfloat16
    AF = mybir.ActivationFunctionType
    Alu = mybir.AluOpType

    nc = bacc.Bacc("TRN2", target_bir_lowering=False, debug=False,
                   num_devices=N_CORES)

    # ---- external I/O (per-core shards staged by the host) ----
    d_xt = nc.dram_tensor("xt", [NJ, NK, 128, TCOL], bf16, kind="ExternalInput").ap()
    d_xta = nc.dram_tensor("xta", [NK, 128, TCA], bf16, kind="ExternalInput").ap()
    d_wq = nc.dram_tensor("wq", [HIDDEN, 384], bf16, kind="ExternalInput").ap()
    d_kva = nc.dram_tensor("kva", [HIDDEN, KV_RANK + D_ROPE], bf16,
                           kind="ExternalInput").ap()
    d_kvb = nc.dram_tensor("kvb", [KV_RANK, 512], bf16, kind="ExternalInput").ap()
    d_ws1 = nc.dram_tensor("wssq1", [HIDDEN, 128], bf16, kind="ExternalInput").ap()
    d_ws2 = nc.dram_tensor("wssq2", [HIDDEN, 64], bf16, kind="ExternalInput").ap()
    d_ow = nc.dram_tensor("ow", [HPC * D_V, HIDDEN], bf16, kind="ExternalInput").ap()
    d_cos = nc.dram_tensor("cosT", [128, T], bf16, kind="ExternalInput").ap()
    d_sin = nc.dram_tensor("sinT", [128, T], bf16, kind="ExternalInput").ap()
    d_msw = nc.dram_tensor("mswT", [128, 128], bf16, kind="ExternalInput").ap()
    d_mask = nc.dram_tensor("maskbig", [128, 896], bf16, kind="ExternalInput").ap()
    d_id = nc.dram_tensor("ident", [128, 128], f32, kind="ExternalInput").ap()
    d_out = nc.dram_tensor("out", [NJ, NK, 128, TCOL], bf16,
                           kind="ExternalOutput").ap()

    from contextlib import ExitStack

    with tile.TileContext(nc) as tc, ExitStack() as stk:
        wp = stk.enter_context(tc.tile_pool(name="weights", bufs=1))
        xt_p = stk.enter_context(tc.tile_pool(name="xtp", bufs=2))
        ap_ = stk.enter_context(tc.tile_pool(name="acts", bufs=1))
        sq_p = stk.enter_context(tc.tile_pool(name="sq", bufs=2))
        es_p = stk.enter_context(tc.tile_pool(name="es", bufs=8))
        esm_p = stk.enter_context(tc.tile_pool(name="esum", bufs=2))
        rp = stk.enter_context(tc.tile_pool(name="rope", bufs=1))
        rd_p = stk.enter_context(tc.tile_pool(name="rdp", bufs=2))
        bc_p = stk.enter_context(tc.tile_pool(name="bcast", bufs=1))
        vt_p = stk.enter_context(tc.tile_pool(name="vtmp", bufs=2))
        o_p = stk.enter_context(tc.tile_pool(name="ocopy", bufs=1))
        dram_p = stk.enter_context(tc.tile_pool(name="dram", bufs=1, space="DRAM"))
        pp = stk.enter_context(tc.tile_pool(name="pp", bufs=2, space="PSUM"))
        ps_p = stk.enter_context(tc.tile_pool(name="ps", bufs=2, space="PSUM"))
        pa_p = stk.enter_context(tc.tile_pool(name="pa", bufs=2, space="PSUM"))
        pd_p = stk.enter_context(tc.tile_pool(name="pd", bufs=2, space="PSUM"))

        # ---- resident tiles ----
        ws1 = wp.tile([128, NK, 128], bf16)
        ws2 = wp.tile([128, NK, 64], bf16)
        wq = wp.tile([128, NK, 384], bf16)

        kvb = wp.tile([128, NL, 512], bf16)
        ow = wp.tile([128, HPC, HIDDEN], bf16)
        cosT = wp.tile([128, T], bf16)
        sinT = wp.tile([128, T], bf16)
        mswT = wp.tile([128, 128], bf16)
        maskb = wp.tile([128, 896], bf16)
        ident_f32 = wp.tile([128, 128], f32)
        ones = wp.tile([128, 1], bf16)
        # stage-A tensors share the xt stream ring (they are dead after
        # ~30us; ring deps serialize xt col 0/1 loads behind stage A reads)
        kva = xt_p.tile([128, NK, KV_RANK + D_ROPE], bf16, tag="xt",
                        name="kva")
        xta = xt_p.tile([128, NK, KV_RANK + D_ROPE], bf16, tag="xt",
                        name="xta")

        # activations (feature-major / transposed layouts)
        qn = [ap_.tile([128, T], bf16, tag=f"qn{h}", name=f"qn{h}")
              for h in range(HPC)]
        qpe = ap_.tile([128, T], bf16)          # h0 rows 0:64, h1 rows 64:128
        kn = [ap_.tile([128, T], bf16, tag=f"kn{h}", name=f"kn{h}")
              for h in range(HPC)]
        kpe = ap_.tile([128, T], bf16)          # duplicated in both 64-halves
        latT = ap_.tile([128, NL, T], bf16)
        vna = [ap_.tile([128, NS, D_V], bf16, tag=f"v{h}", name=f"v{h}")
               for h in range(HPC)]
        att = [ap_.tile([128, T], bf16, tag=f"att{h}", name=f"att{h}")
               for h in range(HPC)]
        lat_sb = ap_.tile([128, NL, TCA], bf16)
        ch4 = ap_.tile([128, TCA], bf16)
        ssqrow_q = ap_.tile([1, T], bf16)
        ssqa_q = ap_.tile([1, T], bf16)
        ssqkv = ap_.tile([1, T], bf16)
        srow_q = ap_.tile([1, T], f32)          # rsqrt'ed scales (row layout)
        srow_kv = ap_.tile([1, T], f32)
        skvcol_raw = ap_.tile([128, NS], bf16)
        skvcol = ap_.tile([128, NS], f32)

        # ---- initial DMA loads, most-urgent first per queue ----
        # stage A inputs lead both HWDGE queues, k-interleaved so the first
        # matmuls can start after ~1MB instead of the whole first wave
        dxta = d_xta.rearrange("k p t -> p k t")
        dkva = d_kva.rearrange("(k p) c -> p k c", p=128)
        for g in range(4):
            gs = slice(4 * g, 4 * g + 4)
            q = nc.sync if g < 2 else nc.scalar
            q.dma_start(kva[:, gs, :], dkva[:, gs, :])
            nc.sync.dma_start(xta[:, gs, 0:TCA], dxta[:, gs, :])
        dwq = d_wq.rearrange("(k p) c -> p k c", p=128)
        for g in range(4):
            gs = slice(4 * g, 4 * g + 4)
            nc.gpsimd.dma_start(wq[:, gs, :], dwq[:, gs, :])
        nc.gpsimd.memset(ones[:], 1.0)
        # scalar queue: ssq weights, then stage-B weights + small tables
        dws1 = d_ws1.rearrange("(k p) c -> p k c", p=128)
        dws2 = d_ws2.rearrange("(k p) c -> p k c", p=128)
        for g in range(2):
            gs = slice(8 * g, 8 * g + 8)
            nc.scalar.dma_start(ws1[:, gs, :], dws1[:, gs, :])
            nc.scalar.dma_start(ws2[:, gs, :], dws2[:, gs, :])
        nc.scalar.dma_start(cosT[:], d_cos[:])
        nc.scalar.dma_start(sinT[:], d_sin[:])
        nc.scalar.dma_start(mswT[:], d_msw[:])

        # ---- stage A: kv latent + kv-ssq for this core's 256 tokens ----
        dqa = pd_p.tile([1, TCOL], f32, tag="den", name="dqa")
        for g in range(NL):
            pl = pp.tile([128, TCOL], f32, tag="proj", name=f"latA{g}")
            for k in range(NK):
                nc.tensor.matmul(pl[:, 0:TCA], kva[:, k, 128 * g:128 * g + 128],
                                 xta[:, k, 0:TCA], start=(k == 0),
                                 stop=(k == NK - 1))
            sqa = sq_p.tile([128, TCOL], bf16, tag="sq", name=f"sqa{g}")
            nc.scalar.activation(sqa[:, 0:TCA], pl[:, 0:TCA], AF.Square)
            nc.vector.tensor_copy(lat_sb[:, g, :], pl[:, 0:TCA])
            nc.tensor.matmul(dqa[:, 0:TCA], ones[:, :], sqa[:, 0:TCA],
                             start=(g == 0), stop=(g == NL - 1),
                             skip_group_check=True)
            if g == NL - 1:
                nc.vector.tensor_copy(ch4[64:65, :], dqa[:, 0:TCA])
        plk = pp.tile([128, TCOL], f32, tag="proj", name="latApe")
        for k in range(NK):
            nc.tensor.matmul(plk[0:64, 0:TCA], kva[:, k, KV_RANK:],
                             xta[:, k, 0:TCA], start=(k == 0),
                             stop=(k == NK - 1))
        nc.vector.tensor_copy(ch4[0:64, :], plk[0:64, 0:TCA])

        cc_in = dram_p.tile([5, 128, TCA], bf16)
        cc_out = dram_p.tile([N_CORES, 5, 128, TCA], bf16, addr_space="Shared")
        nc.scalar.dma_start(cc_in[0:NL].rearrange("c p t -> p c t"), lat_sb[:])
        nc.scalar.dma_start(cc_in[NL], ch4[:])
        nc.gpsimd.collective_compute(
            "AllGather", Alu.bypass,
            replica_groups=[list(range(N_CORES))],
            ins=[cc_in.opt()], outs=[cc_out.opt()],
        )
        # ---- rope helper (src: AP of shape [rows, TCOL]) ----
        def rope(dst, src, rows, c):
            e = rp.tile([128, TCOL], f32, tag="re")
            f = rp.tile([128, TCOL], bf16, tag="rf")
            nc.vector.tensor_tensor(e[0:rows, :], src,
                                    cosT[0:rows, c], Alu.mult)
            nc.vector.tensor_tensor(f[0:rows, :], src,
                                    sinT[0:rows, c], Alu.mult)
            pr = ps_p.tile([128, TCOL], f32, tag="score")
            nc.tensor.matmul(pr[0:rows, :], mswT[0:rows, 0:rows], f[0:rows, :],
                             start=True, stop=True)
            nc.vector.tensor_tensor(dst[0:rows, c], e[0:rows, :], pr[0:rows, :],
                                    Alu.add)

        # ---- merged pass: q-ssq + fused q projections (single xt stream) ----
        ar_ins = [dram_p.tile([1, 2 * TCOL], bf16, name=f"ari{x}")
                  for x in range(2)]
        ar_outs = [dram_p.tile([1, 2 * TCOL], bf16, name=f"aro{x}")
                   for x in range(2)]
        for j in range(NJ):
            c = slice(TCOL * j, TCOL * (j + 1))
            xtj = xt_p.tile([128, NK, KV_RANK + D_ROPE], bf16, tag="xt",
                            name=f"xt{j}")
            for g in range(4):
                nc.sync.dma_start(
                    xtj[:, 4 * g:4 * g + 4, 0:TCOL],
                    d_xt[j, 4 * g:4 * g + 4].rearrange("k p t -> p k t"))
            # q-ssq shard
            p0 = pp.tile([128, TCOL], f32, tag="proj")
            for k in range(NK):
                nc.tensor.matmul(p0[:], ws1[:, k, :], xtj[:, k, 0:TCOL],
                                 start=(k == 0), stop=(k == NK - 1))
            s0 = sq_p.tile([128, TCOL], bf16, tag="sq")
            nc.scalar.activation(s0[:], p0[:], AF.Square)
            p1 = pp.tile([128, TCOL], f32, tag="proj")
            for k in range(NK):
                nc.tensor.matmul(p1[0:64, :], ws2[:, k, :], xtj[:, k, 0:TCOL],
                                 start=(k == 0), stop=(k == NK - 1))
            s1 = sq_p.tile([128, TCOL], bf16, tag="sq")
            nc.scalar.activation(s1[0:64, :], p1[0:64, :], AF.Square)
            dq = pd_p.tile([1, TCOL], f32, tag="den")
            nc.tensor.matmul(dq[:], ones[:, :], s0[:], start=True, stop=False)
            nc.tensor.matmul(dq[:], ones[0:64, :], s1[0:64, :],
                             start=False, stop=True)
            nc.vector.tensor_copy(ssqrow_q[0:1, c], dq[:])
            # fused q projections
            for h in range(HPC):
                p = pp.tile([128, TCOL], f32, tag="proj")
                for k in range(NK):
                    nc.tensor.matmul(p[:], wq[:, k, 128 * h:128 * h + 128],
                                     xtj[:, k, 0:TCOL],
                                     start=(k == 0), stop=(k == NK - 1))
                nc.vector.tensor_copy(qn[h][:, c], p[:])
            p = pp.tile([128, TCOL], f32, tag="proj")
            for k in range(NK):
                nc.tensor.matmul(p[:], wq[:, k, 256:384], xtj[:, k, 0:TCOL],
                                 start=(k == 0), stop=(k == NK - 1))
            rope(qpe, p[:, :], 128, c)
            # split q-ssq AllReduce: fire after columns 1 and 3
            if j in (1, 3):
                x = j // 2
                half = slice(TCOL * 2 * x, TCOL * 2 * (x + 1))
                nc.gpsimd.dma_start(ar_ins[x][:], ssqrow_q[0:1, half])
                nc.gpsimd.collective_compute(
                    "AllReduce", Alu.add,
                    replica_groups=[list(range(N_CORES))],
                    ins=[ar_ins[x].opt()], outs=[ar_outs[x].opt()],
                )
                nc.gpsimd.dma_start(ssqa_q[0:1, half], ar_outs[x][:])
                # q scale chain for this half
                nc.vector.tensor_scalar(srow_q[0:1, half], ssqa_q[0:1, half],
                                        1.0 / Q_RANK, EPS, Alu.mult, Alu.add)
                nc.vector.reciprocal_approx_fast(srow_q[0:1, half],
                                                 srow_q[0:1, half])
                nc.scalar.activation(srow_q[0:1, half], srow_q[0:1, half],
                                     AF.Sqrt)

        # deferred weights: not needed before ~85us, keep early HBM free
        nc.scalar.dma_start(kvb[:], d_kvb.rearrange("(k p) c -> p k c", p=128))
        nc.scalar.dma_start(ident_f32[:], d_id[:])
        nc.scalar.dma_start(maskb[:], d_mask[:])
        nc.scalar.dma_start(ow[:], d_ow.rearrange("(h p) c -> p h c", p=128))

        # unpack the gathered latent.  Emitted AFTER the merged pass so the
        # AllReduce trigger DMAs outrank these in queue priority; latT goes
        # over the sync HWDGE queue (idle once the xt stream is issued).
        for b in range(N_CORES):
            cb = slice(TCA * b, TCA * (b + 1))
            nc.sync.dma_start(latT[:, :, cb],
                              cc_out[b, 0:NL].rearrange("c p t -> p c t"))
            nc.sync.dma_start(kpe[0:64, cb], cc_out[b, NL, 0:64, :])
            nc.gpsimd.dma_start(ssqkv[0:1, cb], cc_out[b, NL, 64:65, :])
            nc.gpsimd.dma_start(
                skvcol_raw[:, 2 * b:2 * b + 2],
                cc_out[b, NL, 64:65, :].rearrange("o (g p) -> (o p) g", p=128))

        # kv scale chains (hoisted, full width)
        nc.vector.tensor_scalar(skvcol[:], skvcol_raw[:], 1.0 / KV_RANK, EPS,
                                Alu.mult, Alu.add)
        nc.vector.reciprocal_approx_fast(skvcol[:], skvcol[:])
        nc.scalar.activation(skvcol[:], skvcol[:], AF.Sqrt)
        nc.vector.tensor_scalar(srow_kv[:], ssqkv[:], 1.0 / KV_RANK, EPS,
                                Alu.mult, Alu.add)
        nc.vector.reciprocal_approx_fast(srow_kv[:], srow_kv[:])
        nc.scalar.activation(srow_kv[:], srow_kv[:], AF.Sqrt)

        # ---- stage B: kn + v from the gathered latent; kpe rope ----
        for j in range(NJ):
            c = slice(TCOL * j, TCOL * (j + 1))
            for h in range(HPC):
                pk = pp.tile([128, TCOL], f32, tag="proj", name=f"pkn{j}_{h}")
                for g in range(NL):
                    nc.tensor.matmul(pk[:], kvb[:, g, 128 * h:128 * h + 128],
                                     latT[:, g, c], start=(g == 0),
                                     stop=(g == NL - 1))
                nc.vector.tensor_copy(kn[h][:, c], pk[:])
            for h in range(HPC):
                pv = pp.tile([128, TCOL], f32, tag="proj", name=f"pvt{j}_{h}")
                for g in range(NL):
                    nc.tensor.matmul(pv[:],
                                     kvb[:, g, 256 + 128 * h:384 + 128 * h],
                                     latT[:, g, c], start=(g == 0),
                                     stop=(g == NL - 1))
                vts = vt_p.tile([128, TCOL], f32, tag="vt")
                nc.vector.tensor_copy(vts[:], pv[:])
                ptr = ps_p.tile([128, TCOL], f32, tag="score",
                                name=f"vtr{j}_{h}")
                for sl in range(4):
                    si = 4 * j + sl
                    nc.tensor.transpose(ptr[:, 128 * sl:128 * sl + 128],
                                        vts[:, 128 * sl:128 * sl + 128],
                                        ident_f32[:])
                    nc.vector.tensor_scalar_mul(
                        vna[h][:, si, :], ptr[:, 128 * sl:128 * sl + 128],
                        skvcol[:, si:si + 1])
            rope(kpe, kpe[0:64, c], 64, c)
            nc.sync.dma_start(kpe[64:128, c], kpe[0:64, c])

        # o_proj helpers: one m-tile at a time so emission can interleave
        # with the NEXT column's attention, filling PE dependency bubbles.
        # Output staged in 4-tile groups (ring of 2) to cap SBUF use.
        ostate = {}

        def emit_oproj_tile(jp, m):
            cp = slice(TCOL * jp, TCOL * (jp + 1))
            if m % 4 == 0:
                ostate['g'] = o_p.tile([128, 4, TCOL], bf16, tag="ot",
                                       name=f"ot{jp}_{m // 4}")
            po = pp.tile([128, TCOL], f32, tag="proj", name=f"po{jp}_{m}")
            for h in range(HPC):
                nc.tensor.matmul(po[:], ow[:, h, 128 * m:128 * m + 128],
                                 att[h][:, cp],
                                 start=(h == 0), stop=(h == HPC - 1))
            eng = nc.vector if m % 2 == 0 else nc.scalar
            if eng is nc.vector:
                eng.tensor_copy(ostate['g'][:, m % 4, :], po[:])
            else:
                nc.scalar.copy(ostate['g'][:, m % 4, :], po[:])
            if m % 4 == 3:
                doj = d_out[jp].rearrange("m p t -> p m t")
                nc.sync.dma_start(doj[:, m - 3:m + 1, :], ostate['g'][:])

        def emit_oproj_col(jp):
            for m in range(NK):
                emit_oproj_tile(jp, m)

        # ---- per-column: late scaling then attention for both heads ----
        for j in range(NJ):
            c = slice(TCOL * j, TCOL * (j + 1))
            sqB = bc_p.tile([128, TCOL], f32, tag="sqB")
            nc.gpsimd.partition_broadcast(sqB[:], srow_q[0:1, c])
            skvB = bc_p.tile([128, TCOL], f32, tag="skvB")
            nc.gpsimd.partition_broadcast(skvB[:], srow_kv[0:1, c])
            for h in range(HPC):
                nc.vector.tensor_tensor(qn[h][:, c], qn[h][:, c], sqB[:],
                                        Alu.mult)
                nc.vector.tensor_tensor(kn[h][:, c], kn[h][:, c], skvB[:],
                                        Alu.mult)
            nc.vector.tensor_tensor(qpe[:, c], qpe[:, c], sqB[:], Alu.mult)

            # attention in S^T[s, t] layout, causal block-skip, software-
            # pipelined emission (den/PV of step i-1 after scores of step i)
            pa2 = [pa_p.tile([128, TCOL], f32, tag="attn", name=f"pa{j}_{h}")
                   for h in range(HPC)]
            pden2 = [pd_p.tile([1, TCOL], f32, tag="den", name=f"pden{j}_{h}")
                     for h in range(HPC)]
            n_s = 4 * (j + 1)
            es_hist = [[], []]
            om = NK
            if j > 0:
                om = 0
                # warm-up filler for the PE while the scale mults run
                while om < 2:
                    emit_oproj_tile(j - 1, om)
                    om += 1

            def emit_pv(i, h):
                nc.tensor.matmul(pa2[h][:], vna[h][:, i, :], es_hist[h][i][:],
                                 start=(i == 0), stop=(i == n_s - 1),
                                 skip_group_check=True)

            def emit_den_pair(i2, h):
                # den over an es pair pre-summed on DVE: halves the PE's
                # ones-matmul traffic; emitted 2 steps late so the DVE add
                # never heads-of-line-blocks the PE queue
                esum = esm_p.tile([128, TCOL], bf16, tag="esum")
                nc.vector.tensor_tensor(esum[:], es_hist[h][i2 - 1][:],
                                        es_hist[h][i2][:], Alu.add)
                nc.tensor.matmul(pden2[h][:], ones[:, :], esum[:],
                                 start=(i2 == 1), stop=(i2 == n_s - 1),
                                 skip_group_check=True)

            for i in range(n_s):
                for h in range(HPC):
                    st = ps_p.tile([128, TCOL], f32, tag="score")
                    nc.tensor.matmul(st[:], kn[h][:, 128 * i:128 * i + 128],
                                     qn[h][:, c], start=True, stop=False)
                    nc.tensor.matmul(st[:],
                                     kpe[64 * h:64 * h + 64,
                                         128 * i:128 * i + 128],
                                     qpe[64 * h:64 * h + 64, c],
                                     start=False, stop=True)
                    if i > 0:
                        emit_pv(i - 1, h)
                    if i >= 3 and (i - 2) % 2 == 1:
                        emit_den_pair(i - 2, h)
                    if i >= 4 * j:
                        ko = i - 4 * j
                        nc.vector.tensor_tensor(
                            st[:], st[:],
                            maskb[:, 384 - 128 * ko:896 - 128 * ko], Alu.add)
                    es = es_p.tile([128, TCOL], bf16, tag="es")
                    nc.scalar.activation(es[:], st[:], AF.Exp)
                    es_hist[h].append(es)
                # spread previous column's o_proj through this column's
                # attention as ready PE work
                while om < min(NK, (i + 1) * NK // n_s + 2):
                    emit_oproj_tile(j - 1, om)
                    om += 1
            while om < NK:
                emit_oproj_tile(j - 1, om)
                om += 1
            for h in range(HPC):
                emit_pv(n_s - 1, h)
            for h in range(HPC):
                emit_den_pair(n_s - 1, h)
            for h in range(HPC):
                rden = rd_p.tile([1, TCOL], f32, tag="rden")
                nc.vector.reciprocal_approx_fast(rden[:], pden2[h][:])
                rdB = rd_p.tile([128, TCOL], f32, tag="rdB")
                nc.gpsimd.partition_broadcast(rdB[:], rden[:])
                nc.vector.tensor_tensor(att[h][:, c], pa2[h][:], rdB[:],
                                        Alu.mult)

        # o_proj for the last column (earlier columns were emitted inside
        # the following column's attention loop as PE filler work)
        emit_oproj_col(NJ - 1)

    nc.compile()
    return nc


def _host_prep(positions, hidden_states, q_a_w, q_a_ln_w, q_b_w,
               kv_a_w, kv_a_ln_w, kv_b_w, o_w):
    pos = np.asarray(positions, dtype=np.float32)
    hs = np.asarray(hidden_states, dtype=np.float32)
    q_a_w = np.asarray(q_a_w, dtype=np.float32)
    q_b_w = np.asarray(q_b_w, dtype=np.float32) * np.asarray(
        q_a_ln_w, dtype=np.float32)[:, None]
    kv_a_w = np.asarray(kv_a_w, dtype=np.float32)
    kv_b_w = np.asarray(kv_b_w, dtype=np.float32) * np.asarray(
        kv_a_ln_w, dtype=np.float32)[:, None]
    o_w = np.asarray(o_w, dtype=np.float32)

    # fused q weights (softmax scale folded in)
    wq_full = (q_a_w @ q_b_w).reshape(HIDDEN, H, QK_DIM) * SCALE
    kvb = kv_b_w.reshape(KV_RANK, H, D_NOPE + D_V)

    # rope pair permutation: interleaved (0::2, 1::2) -> (x1 block | x2 block)
    qpe_cols = wq_full[:, :, D_NOPE:]
    qpe_perm = np.concatenate([qpe_cols[:, :, 0::2], qpe_cols[:, :, 1::2]],
                              axis=2)  # [HIDDEN, H, 64]
    kva_perm = kv_a_w.copy()
    wkpe = kv_a_w[:, KV_RANK:]
    kva_perm[:, KV_RANK:] = np.concatenate([wkpe[:, 0::2], wkpe[:, 1::2]],
                                           axis=1)

    inv_freq = 1.0 / (ROPE_BASE ** (np.arange(0, D_ROPE, 2,
                                              dtype=np.float32) / D_ROPE))
    freqs = pos[None, :] * inv_freq[:, None]           # [32, T]
    cosT = np.tile(np.cos(freqs), (4, 1)).astype(BF16)   # [128, T]
    sinT = np.tile(np.sin(freqs), (4, 1)).astype(BF16)

    # band-swap-with-sign matrix: o = e + Msw @ f
    msw = np.zeros((128, 128), dtype=np.float32)
    for q in range(2):
        for i in range(32):
            msw[64 * q + i, 64 * q + 32 + i] = -1.0
            msw[64 * q + 32 + i, 64 * q + i] = 1.0
    mswT = np.ascontiguousarray(msw.T).astype(BF16)

    # big causal mask: maskb[s, col] = 0 if col >= s + 384 else NEG
    col = np.arange(896)[None, :]
    s_ = np.arange(128)[:, None]
    maskb = np.where(col >= s_ + 384, 0.0, NEG).astype(BF16)

    hsT = np.ascontiguousarray(hs.T).astype(BF16)      # [HIDDEN, T]
    xt = np.ascontiguousarray(
        hsT.reshape(NK, 128, NJ, TCOL).transpose(2, 0, 1, 3))
    xta_all = hsT.reshape(NK, 128, N_CORES, TCA)

    ident = np.eye(128, dtype=np.float32)

    in_maps = []
    for cidx in range(N_CORES):
        h0 = HPC * cidx
        wq_c = np.concatenate(
            [wq_full[:, h0 + h, :D_NOPE] for h in range(HPC)]
            + [qpe_perm[:, h0 + h, :] for h in range(HPC)], axis=1)
        kvb_c = np.concatenate(
            [kvb[:, h0 + h, :D_NOPE] for h in range(HPC)]
            + [kvb[:, h0 + h, D_NOPE:] for h in range(HPC)], axis=1)
        ws1 = q_a_w[:, 192 * cidx:192 * cidx + 128]
        ws2 = q_a_w[:, 192 * cidx + 128:192 * (cidx + 1)]
        ow_c = o_w[D_V * h0:D_V * (h0 + HPC), :]
        in_maps.append({
            "xt": xt,
            "xta": np.ascontiguousarray(xta_all[:, :, cidx, :]),
            "wq": np.ascontiguousarray(wq_c).astype(BF16),
            "kva": np.ascontiguousarray(kva_perm).astype(BF16),
            "kvb": np.ascontiguousarray(kvb_c).astype(BF16),
            "wssq1": np.ascontiguousarray(ws1).astype(BF16),
            "wssq2": np.ascontiguousarray(ws2).astype(BF16),
            "ow": np.ascontiguousarray(ow_c).astype(BF16),
            "cosT": cosT,
            "sinT": sinT,
            "mswT": mswT,
            "maskbig": maskb,
            "ident": ident,
        })
    return in_maps


def kernel(**inputs):
    from concourse.bass_utils import run_bass_kernel_spmd

    if "nc" not in _CACHE:
        _CACHE["nc"] = _build_program()
    nc = _CACHE["nc"]

    in_maps = _host_prep(**inputs)
    trace = bool(int(os.environ.get("BASSK_TRACE", "0")))
    tmpdir = os.environ.get("BASSK_TMPDIR") or None
    if tmpdir:
        os.makedirs(tmpdir, exist_ok=True)
    res = run_bass_kernel_spmd(nc, in_maps, core_ids=list(range(N_CORES)),
                               trace=trace, tmpdir=tmpdir)
    _CACHE["last_exec_time_ns"] = res.exec_time_ns
    _CACHE["last_results"] = res.results
    outT = np.zeros((NJ, NK, 128, TCOL), dtype=np.float32)
    for r in res.results:
        outT += np.asarray(r["out"], dtype=np.float32)
    outT = outT.transpose(1, 2, 0, 3).reshape(HIDDEN, T)
    return np.ascontiguousarray(outT.T)


# revision 17
# speedup vs baseline: 1.0599x; 1.0207x over previous
"""DeepseekV2 MLA attention on 8 TRN2 NeuronCores (Bass/Tile), v2.

Strategy (tensor-parallel over heads, 2 heads/core), changes vs v1:
  - KV path is two-stage: stage A computes the full 576-dim kv_a latent for
    this core's own 256 tokens (dedicated xta input slice), one AllGather
    (~21us) shares the latent + per-token kv ssq across cores, stage B
    (latent @ kv_b, 512-contraction) replaces the fused wk/wv/wkpe
    projections (2048-contraction): ~120k fewer PE columns per core.
  - V comes out of stage B feature-major; PE-transposed into natural [s, dv]
    tiles with the RMSNorm scale fused into the per-partition copy.
  - The q-ssq pass shares the single xt stream with the wq projections
    (xt streamed once, not twice); its AllReduce is split in two so early
    attention columns unblock sooner.
  - All reciprocals via reciprocal_approx_fast (5x; the [1,512]
    single-partition reciprocals were 3.3us each on DVE).
  - RoPE pairs host-permuted to block layout; softmax scale folded into Wq;
    RMSNorm ln weights folded into the following projections (exact).
  - Attention in S^T[s,t] layout: scores via PE (k stationary), exp on ACT
    (no max subtraction; |scores| ~ O(1)), causal masking via additive mask
    on diagonal tiles, denominator via ones-vector matmul, PV with natural-
    layout V stationary.  o_proj row-parallel, host sums 8 partials.
"""

import os
import sys

import numpy as np

for _p in ("/opt/trn_rl_repo",):
    if _p not in sys.path and os.path.isdir(_p):
        sys.path.insert(0, _p)

import ml_dtypes  # noqa: E402

BF16 = ml_dtypes.bfloat16

H = 16
D_NOPE = 128
D_ROPE = 64
D_V = 128
KV_RANK = 512
Q_RANK = 1536
HIDDEN = 2048
T = 2048
EPS = 1e-6
QK_DIM = D_NOPE + D_ROPE
SCALE = QK_DIM ** -0.5
ROPE_BASE = 10000.0

N_CORES = 8
HPC = H // N_CORES          # heads per core = 2
TCOL = 512                  # moving-operand width
TCA = 256                   # stage-A token slice per core
NJ = T // TCOL              # 4 t-column blocks
NK = HIDDEN // 128          # 16 contraction chunks
NL = KV_RANK // 128         # 4 latent chunks
NS = T // 128               # 16 key tiles
NEG = -1.0e4                # causal mask additive value

_CACHE = {}


def _build_program():
    import concourse.bass as bass  # noqa: F401
    import concourse.mybir as mybir
    import concourse.tile as tile
    from concourse import bacc

    f32 = mybir.dt.float32
    bf16 = mybir.dt.bfloat16
    AF = mybir.ActivationFunctionType
    Alu = mybir.AluOpType

    nc = bacc.Bacc("TRN2", target_bir_lowering=False, debug=False,
                   num_devices=N_CORES)

    # ---- external I/O (per-core shards staged by the host) ----
    d_xt = nc.dram_tensor("xt", [NJ, NK, 128, TCOL], bf16, kind="ExternalInput").ap()
    d_xta = nc.dram_tensor("xta", [NK, 128, TCA], bf16, kind="ExternalInput").ap()
    d_wq = nc.dram_tensor("wq", [HIDDEN, 384], bf16, kind="ExternalInput").ap()
    d_kva = nc.dram_tensor("kva", [HIDDEN, KV_RANK + D_ROPE], bf16,
                           kind="ExternalInput").ap()
    d_kvb = nc.dram_tensor("kvb", [KV_RANK, 512], bf16, kind="ExternalInput").ap()
    d_ws1 = nc.dram_tensor("wssq1", [HIDDEN, 128], bf16, kind="ExternalInput").ap()
    d_ws2 = nc.dram_tensor("wssq2", [HIDDEN, 64], bf16, kind="ExternalInput").ap()
    d_ow = nc.dram_tensor("ow", [HPC * D_V, HIDDEN], bf16, kind="ExternalInput").ap()
    d_cos = nc.dram_tensor("cosT", [128, T], bf16, kind="ExternalInput").ap()
    d_sin = nc.dram_tensor("sinT", [128, T], bf16, kind="ExternalInput").ap()
    d_msw = nc.dram_tensor("mswT", [128, 128], bf16, kind="ExternalInput").ap()
    d_mask = nc.dram_tensor("maskbig", [128, 896], bf16, kind="ExternalInput").ap()
    d_id = nc.dram_tensor("ident", [128, 128], f32, kind="ExternalInput").ap()
    d_out = nc.dram_tensor("out", [NJ, NK, 128, TCOL], bf16,
                           kind="ExternalOutput").ap()

    from contextlib import ExitStack

    with tile.TileContext(nc) as tc, ExitStack() as stk:
        wp = stk.enter_context(tc.tile_pool(name="weights", bufs=1))
        xt_p = stk.enter_context(tc.tile_pool(name="xtp", bufs=2))
        ap_ = stk.enter_context(tc.tile_pool(name="acts", bufs=1))
        sq_p = stk.enter_context(tc.tile_pool(name="sq", bufs=2))
        es_p = stk.enter_context(tc.tile_pool(name="es", bufs=8))
        esm_p = stk.enter_context(tc.tile_pool(name="esum", bufs=2))
        rp = stk.enter_context(tc.tile_pool(name="rope", bufs=1))
        rd_p = stk.enter_context(tc.tile_pool(name="rdp", bufs=2))
        bc_p = stk.enter_context(tc.tile_pool(name="bcast", bufs=1))
        vt_p = stk.enter_context(tc.tile_pool(name="vtmp", bufs=2))
        o_p = stk.enter_context(tc.tile_pool(name="ocopy", bufs=1))
        dram_p = stk.enter_context(tc.tile_pool(name="dram", bufs=1, space="DRAM"))
        pp = stk.enter_context(tc.tile_pool(name="pp", bufs=2, space="PSUM"))
        ps_p = stk.enter_context(tc.tile_pool(name="ps", bufs=2, space="PSUM"))
        pa_p = stk.enter_context(tc.tile_pool(name="pa", bufs=2, space="PSUM"))
        pd_p = stk.enter_context(tc.tile_pool(name="pd", bufs=2, space="PSUM"))

        # ---- resident tiles ----
        ws1 = wp.tile([128, NK, 128], bf16)
        ws2 = wp.tile([128, NK, 64], bf16)
        wq = wp.tile([128, NK, 384], bf16)

        kvb = wp.tile([128, NL, 512], bf16)
        ow = wp.tile([128, HPC, HIDDEN], bf16)
        cosT = wp.tile([128, T], bf16)
        sinT = wp.tile([128, T], bf16)
        mswT = wp.tile([128, 128], bf16)
        maskb = wp.tile([128, 896], bf16)
        ident_f32 = wp.tile([128, 128], f32)
        ones = wp.tile([128, 1], bf16)
        # stage-A tensors share the xt stream ring (they are dead after
        # ~30us; ring deps serialize xt col 0/1 loads behind stage A reads)
        kva = xt_p.tile([128, NK, KV_RANK + D_ROPE], bf16, tag="xt",
                        name="kva")
        xta = xt_p.tile([128, NK, KV_RANK + D_ROPE], bf16, tag="xt",
                        name="xta")

        # activations (feature-major / transposed layouts)
        qn = [ap_.tile([128, T], bf16, tag=f"qn{h}", name=f"qn{h}")
              for h in range(HPC)]
        qpe = ap_.tile([128, T], bf16)          # h0 rows 0:64, h1 rows 64:128
        kn = [ap_.tile([128, T], bf16, tag=f"kn{h}", name=f"kn{h}")
              for h in range(HPC)]
        kpe = ap_.tile([128, T], bf16)          # duplicated in both 64-halves
        latT = ap_.tile([128, NL, T], bf16)
        vna = [ap_.tile([128, NS, D_V], bf16, tag=f"v{h}", name=f"v{h}")
               for h in range(HPC)]
        att = [ap_.tile([128, T], bf16, tag=f"att{h}", name=f"att{h}")
               for h in range(HPC)]
        lat_sb = ap_.tile([128, NL, TCA], bf16)
        ch4 = ap_.tile([128, TCA], bf16)
        ssqrow_q = ap_.tile([1, T], bf16)
        ssqa_q = ap_.tile([1, T], bf16)
        ssqkv = ap_.tile([1, T], bf16)
        srow_q = ap_.tile([1, T], f32)          # rsqrt'ed scales (row layout)
        srow_kv = ap_.tile([1, T], f32)
        skvcol_raw = ap_.tile([128, NS], bf16)
        skvcol = ap_.tile([128, NS], f32)

        # ---- initial DMA loads, most-urgent first per queue ----
        # stage A inputs lead both HWDGE queues, k-interleaved so the first
        # matmuls can start after ~1MB instead of the whole first wave
        dxta = d_xta.rearrange("k p t -> p k t")
        dkva = d_kva.rearrange("(k p) c -> p k c", p=128)
        for g in range(4):
            gs = slice(4 * g, 4 * g + 4)
            q = nc.sync if g < 2 else nc.scalar
            q.dma_start(kva[:, gs, :], dkva[:, gs, :])
            nc.sync.dma_start(xta[:, gs, 0:TCA], dxta[:, gs, :])
        dwq = d_wq.rearrange("(k p) c -> p k c", p=128)
        for g in range(4):
            gs = slice(4 * g, 4 * g + 4)
            nc.gpsimd.dma_start(wq[:, gs, :], dwq[:, gs, :])
        nc.gpsimd.memset(ones[:], 1.0)
        # scalar queue: ssq weights, then stage-B weights + small tables
        dws1 = d_ws1.rearrange("(k p) c -> p k c", p=128)
        dws2 = d_ws2.rearrange("(k p) c -> p k c", p=128)
        for g in range(2):
            gs = slice(8 * g, 8 * g + 8)
            nc.scalar.dma_start(ws1[:, gs, :], dws1[:, gs, :])
            nc.scalar.dma_start(ws2[:, gs, :], dws2[:, gs, :])
        nc.scalar.dma_start(cosT[:], d_cos[:])
        nc.scalar.dma_start(sinT[:], d_sin[:])
        nc.scalar.dma_start(mswT[:], d_msw[:])

        # ---- stage A: kv latent + kv-ssq for this core's 256 tokens ----
        dqa = pd_p.tile([1, TCOL], f32, tag="den", name="dqa")
        for g in range(NL):
            pl = pp.tile([128, TCOL], f32, tag="proj", name=f"latA{g}")
            for k in range(NK):
                nc.tensor.matmul(pl[:, 0:TCA], kva[:, k, 128 * g:128 * g + 128],
                                 xta[:, k, 0:TCA], start=(k == 0),
                                 stop=(k == NK - 1))
            sqa = sq_p.tile([128, TCOL], bf16, tag="sq", name=f"sqa{g}")
            nc.scalar.activation(sqa[:, 0:TCA], pl[:, 0:TCA], AF.Square)
            nc.vector.tensor_copy(lat_sb[:, g, :], pl[:, 0:TCA])
            nc.tensor.matmul(dqa[:, 0:TCA], ones[:, :], sqa[:, 0:TCA],
                             start=(g == 0), stop=(g == NL - 1),
                             skip_group_check=True)
            if g == NL - 1:
                nc.vector.tensor_copy(ch4[64:65, :], dqa[:, 0:TCA])
        plk = pp.tile([128, TCOL], f32, tag="proj", name="latApe")
        for k in range(NK):
            nc.tensor.matmul(plk[0:64, 0:TCA], kva[:, k, KV_RANK:],
                             xta[:, k, 0:TCA], start=(k == 0),
                             stop=(k == NK - 1))
        nc.vector.tensor_copy(ch4[0:64, :], plk[0:64, 0:TCA])

        cc_in = dram_p.tile([5, 128, TCA], bf16)
        cc_out = dram_p.tile([N_CORES, 5, 128, TCA], bf16, addr_space="Shared")
        nc.scalar.dma_start(cc_in[0:NL].rearrange("c p t -> p c t"), lat_sb[:])
        nc.scalar.dma_start(cc_in[NL], ch4[:])
        nc.gpsimd.collective_compute(
            "AllGather", Alu.bypass,
            replica_groups=[list(range(N_CORES))],
            ins=[cc_in.opt()], outs=[cc_out.opt()],
        )
        # ---- rope helper (src: AP of shape [rows, TCOL]) ----
        def rope(dst, src, rows, c):
            e = rp.tile([128, TCOL], f32, tag="re")
            f = rp.tile([128, TCOL], bf16, tag="rf")
            nc.vector.tensor_tensor(e[0:rows, :], src,
                                    cosT[0:rows, c], Alu.mult)
            nc.vector.tensor_tensor(f[0:rows, :], src,
                                    sinT[0:rows, c], Alu.mult)
            pr = ps_p.tile([128, TCOL], f32, tag="score")
            nc.tensor.matmul(pr[0:rows, :], mswT[0:rows, 0:rows], f[0:rows, :],
                             start=True, stop=True)
            nc.vector.tensor_tensor(dst[0:rows, c], e[0:rows, :], pr[0:rows, :],
                                    Alu.add)

        # ---- merged pass: q-ssq + fused q projections (single xt stream) ----
        ar_ins = [dram_p.tile([1, 2 * TCOL], bf16, name=f"ari{x}")
                  for x in range(2)]
        ar_outs = [dram_p.tile([1, 2 * TCOL], bf16, name=f"aro{x}")
                   for x in range(2)]
        for j in range(NJ):
            c = slice(TCOL * j, TCOL * (j + 1))
            xtj = xt_p.tile([128, NK, KV_RANK + D_ROPE], bf16, tag="xt",
                            name=f"xt{j}")
            for g in range(4):
                nc.sync.dma_start(
                    xtj[:, 4 * g:4 * g + 4, 0:TCOL],
                    d_xt[j, 4 * g:4 * g + 4].rearrange("k p t -> p k t"))
            # q-ssq shard
            p0 = pp.tile([128, TCOL], f32, tag="proj")
            for k in range(NK):
                nc.tensor.matmul(p0[:], ws1[:, k, :], xtj[:, k, 0:TCOL],
                                 start=(k == 0), stop=(k == NK - 1))
            s0 = sq_p.tile([128, TCOL], bf16, tag="sq")
            nc.scalar.activation(s0[:], p0[:], AF.Square)
            p1 = pp.tile([128, TCOL], f32, tag="proj")
            for k in range(NK):
                nc.tensor.matmul(p1[0:64, :], ws2[:, k, :], xtj[:, k, 0:TCOL],
                                 start=(k == 0), stop=(k == NK - 1))
            s1 = sq_p.tile([128, TCOL], bf16, tag="sq")
            nc.scalar.activation(s1[0:64, :], p1[0:64, :], AF.Square)
            dq = pd_p.tile([1, TCOL], f32, tag="den")
            nc.tensor.matmul(dq[:], ones[:, :], s0[:], start=True, stop=False)
            nc.tensor.matmul(dq[:], ones[0:64, :], s1[0:64, :],
                             start=False, stop=True)
            nc.vector.tensor_copy(ssqrow_q[0:1, c], dq[:])
            # fused q projections
            for h in range(HPC):
                p = pp.tile([128, TCOL], f32, tag="proj")
                for k in range(NK):
                    nc.tensor.matmul(p[:], wq[:, k, 128 * h:128 * h + 128],
                                     xtj[:, k, 0:TCOL],
                                     start=(k == 0), stop=(k == NK - 1))
                nc.vector.tensor_copy(qn[h][:, c], p[:])
            p = pp.tile([128, TCOL], f32, tag="proj")
            for k in range(NK):
                nc.tensor.matmul(p[:], wq[:, k, 256:384], xtj[:, k, 0:TCOL],
                                 start=(k == 0), stop=(k == NK - 1))
            rope(qpe, p[:, :], 128, c)
            # split q-ssq AllReduce: fire after columns 1 and 3
            if j in (1, 3):
                x = j // 2
                half = slice(TCOL * 2 * x, TCOL * 2 * (x + 1))
                nc.gpsimd.dma_start(ar_ins[x][:], ssqrow_q[0:1, half])
                nc.gpsimd.collective_compute(
                    "AllReduce", Alu.add,
                    replica_groups=[list(range(N_CORES))],
                    ins=[ar_ins[x].opt()], outs=[ar_outs[x].opt()],
                )
                nc.gpsimd.dma_start(ssqa_q[0:1, half], ar_outs[x][:])
                # q scale chain for this half
                nc.vector.tensor_scalar(srow_q[0:1, half], ssqa_q[0:1, half],
                                        1.0 / Q_RANK, EPS, Alu.mult, Alu.add)
                nc.vector.reciprocal_approx_fast(srow_q[0:1, half],
                                                 srow_q[0:1, half])
                nc.scalar.activation(srow_q[0:1, half], srow_q[0:1, half],
                                     AF.Sqrt)

        # deferred weights: not needed before ~85us, keep early HBM free
        nc.scalar.dma_start(kvb[:], d_kvb.rearrange("(k p) c -> p k c", p=128))
        nc.scalar.dma_start(ident_f32[:], d_id[:])
        nc.scalar.dma_start(maskb[:], d_mask[:])
        nc.scalar.dma_start(ow[:], d_ow.rearrange("(h p) c -> p h c", p=128))

        # unpack the gathered latent.  Emitted AFTER the merged pass so the
        # AllReduce trigger DMAs outrank these in queue priority; latT goes
        # over the sync HWDGE queue (idle once the xt stream is issued).
        for b in range(N_CORES):
            cb = slice(TCA * b, TCA * (b + 1))
            nc.sync.dma_start(latT[:, :, cb],
                              cc_out[b, 0:NL].rearrange("c p t -> p c t"))
            nc.sync.dma_start(kpe[0:64, cb], cc_out[b, NL, 0:64, :])
            nc.gpsimd.dma_start(ssqkv[0:1, cb], cc_out[b, NL, 64:65, :])
            nc.gpsimd.dma_start(
                skvcol_raw[:, 2 * b:2 * b + 2],
                cc_out[b, NL, 64:65, :].rearrange("o (g p) -> (o p) g", p=128))

        # kv scale chains (hoisted, full width)
        nc.vector.tensor_scalar(skvcol[:], skvcol_raw[:], 1.0 / KV_RANK, EPS,
                                Alu.mult, Alu.add)
        nc.vector.reciprocal_approx_fast(skvcol[:], skvcol[:])
        nc.scalar.activation(skvcol[:], skvcol[:], AF.Sqrt)
        nc.vector.tensor_scalar(srow_kv[:], ssqkv[:], 1.0 / KV_RANK, EPS,
                                Alu.mult, Alu.add)
        nc.vector.reciprocal_approx_fast(srow_kv[:], srow_kv[:])
        nc.scalar.activation(srow_kv[:], srow_kv[:], AF.Sqrt)

        # ---- stage B: kn + v from the gathered latent; kpe rope ----
        for j in range(NJ):
            c = slice(TCOL * j, TCOL * (j + 1))
            for h in range(HPC):
                pk = pp.tile([128, TCOL], f32, tag="proj", name=f"pkn{j}_{h}")
                for g in range(NL):
                    nc.tensor.matmul(pk[:], kvb[:, g, 128 * h:128 * h + 128],
                                     latT[:, g, c], start=(g == 0),
                                     stop=(g == NL - 1))
                nc.vector.tensor_copy(kn[h][:, c], pk[:])
            for h in range(HPC):
                pv = pp.tile([128, TCOL], f32, tag="proj", name=f"pvt{j}_{h}")
                for g in range(NL):
                    nc.tensor.matmul(pv[:],
                                     kvb[:, g, 256 + 128 * h:384 + 128 * h],
                                     latT[:, g, c], start=(g == 0),
                                     stop=(g == NL - 1))
                vts = vt_p.tile([128, TCOL], f32, tag="vt")
                nc.vector.tensor_copy(vts[:], pv[:])
                ptr = ps_p.tile([128, TCOL], f32, tag="score",
                                name=f"vtr{j}_{h}")
                for sl in range(4):
                    si = 4 * j + sl
                    nc.tensor.transpose(ptr[:, 128 * sl:128 * sl + 128],
                                        vts[:, 128 * sl:128 * sl + 128],
                                        ident_f32[:])
                    nc.vector.tensor_scalar_mul(
                        vna[h][:, si, :], ptr[:, 128 * sl:128 * sl + 128],
                        skvcol[:, si:si + 1])
            rope(kpe, kpe[0:64, c], 64, c)
            nc.sync.dma_start(kpe[64:128, c], kpe[0:64, c])

        # o_proj helpers: one m-tile at a time so emission can interleave
        # with the NEXT column's attention, filling PE dependency bubbles.
        # Output staged in 4-tile groups (ring of 2) to cap SBUF use.
        ostate = {}

        def emit_oproj_tile(jp, m):
            cp = slice(TCOL * jp, TCOL * (jp + 1))
            if m % 4 == 0:
                ostate['g'] = o_p.tile([128, 4, TCOL], bf16, tag="ot",
                                       name=f"ot{jp}_{m // 4}")
            # the last column's o_proj runs after attention has drained, so
            # the pa PSUM ring is free: alternate pools for a 4-deep ring
            pool = pa_p if (jp == NJ - 1 and m % 2 == 1) else pp
            tag = "attn" if pool is pa_p else "proj"
            po = pool.tile([128, TCOL], f32, tag=tag, name=f"po{jp}_{m}")
            for h in range(HPC):
                nc.tensor.matmul(po[:], ow[:, h, 128 * m:128 * m + 128],
                                 att[h][:, cp],
                                 start=(h == 0), stop=(h == HPC - 1))
            eng = nc.vector if m % 2 == 0 else nc.scalar
            if eng is nc.vector:
                eng.tensor_copy(ostate['g'][:, m % 4, :], po[:])
            else:
                nc.scalar.copy(ostate['g'][:, m % 4, :], po[:])
            if m % 4 == 3:
                doj = d_out[jp].rearrange("m p t -> p m t")
                nc.sync.dma_start(doj[:, m - 3:m + 1, :], ostate['g'][:])

        def emit_oproj_col(jp):
            for m in range(NK):
                emit_oproj_tile(jp, m)

        # ---- per-column: late scaling then attention for both heads ----
        for j in range(NJ):
            c = slice(TCOL * j, TCOL * (j + 1))
            sqB = bc_p.tile([128, TCOL], f32, tag="sqB")
            nc.gpsimd.partition_broadcast(sqB[:], srow_q[0:1, c])
            skvB = bc_p.tile([128, TCOL], f32, tag="skvB")
            nc.gpsimd.partition_broadcast(skvB[:], srow_kv[0:1, c])
            for h in range(HPC):
                nc.vector.tensor_tensor(qn[h][:, c], qn[h][:, c], sqB[:],
                                        Alu.mult)
                nc.vector.tensor_tensor(kn[h][:, c], kn[h][:, c], skvB[:],
                                        Alu.mult)
            nc.vector.tensor_tensor(qpe[:, c], qpe[:, c], sqB[:], Alu.mult)

            # attention in S^T[s, t] layout, causal block-skip, software-
            # pipelined emission (den/PV of step i-1 after scores of step i)
            pa2 = [pa_p.tile([128, TCOL], f32, tag="attn", name=f"pa{j}_{h}")
                   for h in range(HPC)]
            pden2 = [pd_p.tile([1, TCOL], f32, tag="den", name=f"pden{j}_{h}")
                     for h in range(HPC)]
            n_s = 4 * (j + 1)
            es_hist = [[], []]
            om = NK
            if j > 0:
                om = 0
                # warm-up filler for the PE while the scale mults run
                while om < 2:
                    emit_oproj_tile(j - 1, om)
                    om += 1

            def emit_pv(i, h):
                nc.tensor.matmul(pa2[h][:], vna[h][:, i, :], es_hist[h][i][:],
                                 start=(i == 0), stop=(i == n_s - 1),
                                 skip_group_check=True)

            def emit_den_pair(i2, h):
                # den over an es pair pre-summed on DVE: halves the PE's
                # ones-matmul traffic; emitted 2 steps late so the DVE add
                # never heads-of-line-blocks the PE queue
                esum = esm_p.tile([128, TCOL], bf16, tag="esum")
                nc.vector.tensor_tensor(esum[:], es_hist[h][i2 - 1][:],
                                        es_hist[h][i2][:], Alu.add)
                nc.tensor.matmul(pden2[h][:], ones[:, :], esum[:],
                                 start=(i2 == 1), stop=(i2 == n_s - 1),
                                 skip_group_check=True)

            for i in range(n_s):
                for h in range(HPC):
                    st = ps_p.tile([128, TCOL], f32, tag="score")
                    nc.tensor.matmul(st[:], kn[h][:, 128 * i:128 * i + 128],
                                     qn[h][:, c], start=True, stop=False)
                    nc.tensor.matmul(st[:],
                                     kpe[64 * h:64 * h + 64,
                                         128 * i:128 * i + 128],
                                     qpe[64 * h:64 * h + 64, c],
                                     start=False, stop=True)
                    if i > 0:
                        emit_pv(i - 1, h)
                    if i >= 3 and (i - 2) % 2 == 1:
                        emit_den_pair(i - 2, h)
                    if i >= 4 * j:
                        ko = i - 4 * j
                        nc.vector.tensor_tensor(
                            st[:], st[:],
                            maskb[:, 384 - 128 * ko:896 - 128 * ko], Alu.add)
                    es = es_p.tile([128, TCOL], bf16, tag="es")
                    nc.scalar.activation(es[:], st[:], AF.Exp)
                    es_hist[h].append(es)
                # spread previous column's o_proj through this column's
                # attention as ready PE work
                while om < min(NK, (i + 1) * NK // n_s + 2):
                    emit_oproj_tile(j - 1, om)
                    om += 1
            while om < NK:
                emit_oproj_tile(j - 1, om)
                om += 1
            for h in range(HPC):
                emit_pv(n_s - 1, h)
            for h in range(HPC):
                emit_den_pair(n_s - 1, h)
            for h in range(HPC):
                rden = rd_p.tile([1, TCOL], f32, tag="rden")
                nc.vector.reciprocal_approx_fast(rden[:], pden2[h][:])
                rdB = rd_p.tile([128, TCOL], f32, tag="rdB")
                nc.gpsimd.partition_broadcast(rdB[:], rden[:])
                nc.vector.tensor_tensor(att[h][:, c], pa2[h][:], rdB[:],
                                        Alu.mult)

        # o_proj for the last column (earlier columns were emitted inside
        # the following column's attention loop as PE filler work)
        emit_oproj_col(NJ - 1)

    nc.compile()
    return nc


def _host_prep(positions, hidden_states, q_a_w, q_a_ln_w, q_b_w,
               kv_a_w, kv_a_ln_w, kv_b_w, o_w):
    pos = np.asarray(positions, dtype=np.float32)
    hs = np.asarray(hidden_states, dtype=np.float32)
    q_a_w = np.asarray(q_a_w, dtype=np.float32)
    q_b_w = np.asarray(q_b_w, dtype=np.float32) * np.asarray(
        q_a_ln_w, dtype=np.float32)[:, None]
    kv_a_w = np.asarray(kv_a_w, dtype=np.float32)
    kv_b_w = np.asarray(kv_b_w, dtype=np.float32) * np.asarray(
        kv_a_ln_w, dtype=np.float32)[:, None]
    o_w = np.asarray(o_w, dtype=np.float32)

    # fused q weights (softmax scale folded in)
    wq_full = (q_a_w @ q_b_w).reshape(HIDDEN, H, QK_DIM) * SCALE
    kvb = kv_b_w.reshape(KV_RANK, H, D_NOPE + D_V)

    # rope pair permutation: interleaved (0::2, 1::2) -> (x1 block | x2 block)
    qpe_cols = wq_full[:, :, D_NOPE:]
    qpe_perm = np.concatenate([qpe_cols[:, :, 0::2], qpe_cols[:, :, 1::2]],
                              axis=2)  # [HIDDEN, H, 64]
    kva_perm = kv_a_w.copy()
    wkpe = kv_a_w[:, KV_RANK:]
    kva_perm[:, KV_RANK:] = np.concatenate([wkpe[:, 0::2], wkpe[:, 1::2]],
                                           axis=1)

    inv_freq = 1.0 / (ROPE_BASE ** (np.arange(0, D_ROPE, 2,
                                              dtype=np.float32) / D_ROPE))
    freqs = pos[None, :] * inv_freq[:, None]           # [32, T]
    cosT = np.tile(np.cos(freqs), (4, 1)).astype(BF16)   # [128, T]
    sinT = np.tile(np.sin(freqs), (4, 1)).astype(BF16)

    # band-swap-with-sign matrix: o = e + Msw @ f
    msw = np.zeros((128, 128), dtype=np.float32)
    for q in range(2):
        for i in range(32):
            msw[64 * q + i, 64 * q + 32 + i] = -1.0
            msw[64 * q + 32 + i, 64 * q + i] = 1.0
    mswT = np.ascontiguousarray(msw.T).astype(BF16)

    # big causal mask: maskb[s, col] = 0 if col >= s + 384 else NEG
    col = np.arange(896)[None, :]
    s_ = np.arange(128)[:, None]
    maskb = np.where(col >= s_ + 384, 0.0, NEG).astype(BF16)

    hsT = np.ascontiguousarray(hs.T).astype(BF16)      # [HIDDEN, T]
    xt = np.ascontiguousarray(
        hsT.reshape(NK, 128, NJ, TCOL).transpose(2, 0, 1, 3))
    xta_all = hsT.reshape(NK, 128, N_CORES, TCA)

    ident = np.eye(128, dtype=np.float32)

    in_maps = []
    for cidx in range(N_CORES):
        h0 = HPC * cidx
        wq_c = np.concatenate(
            [wq_full[:, h0 + h, :D_NOPE] for h in range(HPC)]
            + [qpe_perm[:, h0 + h, :] for h in range(HPC)], axis=1)
        kvb_c = np.concatenate(
            [kvb[:, h0 + h, :D_NOPE] for h in range(HPC)]
            + [kvb[:, h0 + h, D_NOPE:] for h in range(HPC)], axis=1)
        ws1 = q_a_w[:, 192 * cidx:192 * cidx + 128]
        ws2 = q_a_w[:, 192 * cidx + 128:192 * (cidx + 1)]
        ow_c = o_w[D_V * h0:D_V * (h0 + HPC), :]
        in_maps.append({
            "xt": xt,
            "xta": np.ascontiguousarray(xta_all[:, :, cidx, :]),
            "wq": np.ascontiguousarray(wq_c).astype(BF16),
            "kva": np.ascontiguousarray(kva_perm).astype(BF16),
            "kvb": np.ascontiguousarray(kvb_c).astype(BF16),
            "wssq1": np.ascontiguousarray(ws1).astype(BF16),
            "wssq2": np.ascontiguousarray(ws2).astype(BF16),
            "ow": np.ascontiguousarray(ow_c).astype(BF16),
            "cosT": cosT,
            "sinT": sinT,
            "mswT": mswT,
            "maskbig": maskb,
            "ident": ident,
        })
    return in_maps


def kernel(**inputs):
    from concourse.bass_utils import run_bass_kernel_spmd

    if "nc" not in _CACHE:
        _CACHE["nc"] = _build_program()
    nc = _CACHE["nc"]

    in_maps = _host_prep(**inputs)
    trace = bool(int(os.environ.get("BASSK_TRACE", "0")))
    tmpdir = os.environ.get("BASSK_TMPDIR") or None
    if tmpdir:
        os.makedirs(tmpdir, exist_ok=True)
    res = run_bass_kernel_spmd(nc, in_maps, core_ids=list(range(N_CORES)),
                               trace=trace, tmpdir=tmpdir)
    _CACHE["last_exec_time_ns"] = res.exec_time_ns
    _CACHE["last_results"] = res.results
    outT = np.zeros((NJ, NK, 128, TCOL), dtype=np.float32)
    for r in res.results:
        outT += np.asarray(r["out"], dtype=np.float32)
    outT = outT.transpose(1, 2, 0, 3).reshape(HIDDEN, T)
    return np.ascontiguousarray(outT.T)


# revision 18
# speedup vs baseline: 1.0649x; 1.0047x over previous
"""DeepseekV2 MLA attention on 8 TRN2 NeuronCores (Bass/Tile), v2.

Strategy (tensor-parallel over heads, 2 heads/core), changes vs v1:
  - KV path is two-stage: stage A computes the full 576-dim kv_a latent for
    this core's own 256 tokens (dedicated xta input slice), one AllGather
    (~21us) shares the latent + per-token kv ssq across cores, stage B
    (latent @ kv_b, 512-contraction) replaces the fused wk/wv/wkpe
    projections (2048-contraction): ~120k fewer PE columns per core.
  - V comes out of stage B feature-major; PE-transposed into natural [s, dv]
    tiles with the RMSNorm scale fused into the per-partition copy.
  - The q-ssq pass shares the single xt stream with the wq projections
    (xt streamed once, not twice); its AllReduce is split in two so early
    attention columns unblock sooner.
  - All reciprocals via reciprocal_approx_fast (5x; the [1,512]
    single-partition reciprocals were 3.3us each on DVE).
  - RoPE pairs host-permuted to block layout; softmax scale folded into Wq;
    RMSNorm ln weights folded into the following projections (exact).
  - Attention in S^T[s,t] layout: scores via PE (k stationary), exp on ACT
    (no max subtraction; |scores| ~ O(1)), causal masking via additive mask
    on diagonal tiles, denominator via ones-vector matmul, PV with natural-
    layout V stationary.  o_proj row-parallel, host sums 8 partials.
"""

import os
import sys

import numpy as np

for _p in ("/opt/trn_rl_repo",):
    if _p not in sys.path and os.path.isdir(_p):
        sys.path.insert(0, _p)

import ml_dtypes  # noqa: E402

BF16 = ml_dtypes.bfloat16

H = 16
D_NOPE = 128
D_ROPE = 64
D_V = 128
KV_RANK = 512
Q_RANK = 1536
HIDDEN = 2048
T = 2048
EPS = 1e-6
QK_DIM = D_NOPE + D_ROPE
SCALE = QK_DIM ** -0.5
ROPE_BASE = 10000.0

N_CORES = 8
HPC = H // N_CORES          # heads per core = 2
TCOL = 512                  # moving-operand width
TCA = 256                   # stage-A token slice per core
NJ = T // TCOL              # 4 t-column blocks
NK = HIDDEN // 128          # 16 contraction chunks
NL = KV_RANK // 128         # 4 latent chunks
NS = T // 128               # 16 key tiles
NEG = -1.0e4                # causal mask additive value

_CACHE = {}


def _build_program():
    import concourse.bass as bass  # noqa: F401
    import concourse.mybir as mybir
    import concourse.tile as tile
    from concourse import bacc

    f32 = mybir.dt.float32
    bf16 = mybir.dt.bfloat16
    AF = mybir.ActivationFunctionType
    Alu = mybir.AluOpType

    nc = bacc.Bacc("TRN2", target_bir_lowering=False, debug=False,
                   num_devices=N_CORES)

    # ---- external I/O (per-core shards staged by the host) ----
    d_xt = nc.dram_tensor("xt", [NJ, NK, 128, TCOL], bf16, kind="ExternalInput").ap()
    d_xta = nc.dram_tensor("xta", [NK, 128, TCA], bf16, kind="ExternalInput").ap()
    d_wq = nc.dram_tensor("wq", [HIDDEN, 384], bf16, kind="ExternalInput").ap()
    d_kva = nc.dram_tensor("kva", [HIDDEN, KV_RANK + D_ROPE], bf16,
                           kind="ExternalInput").ap()
    d_kvb = nc.dram_tensor("kvb", [KV_RANK, 512], bf16, kind="ExternalInput").ap()
    d_ws1 = nc.dram_tensor("wssq1", [HIDDEN, 128], bf16, kind="ExternalInput").ap()
    d_ws2 = nc.dram_tensor("wssq2", [HIDDEN, 64], bf16, kind="ExternalInput").ap()
    d_ow = nc.dram_tensor("ow", [HPC * D_V, HIDDEN], bf16, kind="ExternalInput").ap()
    d_cos = nc.dram_tensor("cosT", [128, T], bf16, kind="ExternalInput").ap()
    d_sin = nc.dram_tensor("sinT", [128, T], bf16, kind="ExternalInput").ap()
    d_msw = nc.dram_tensor("mswT", [128, 128], bf16, kind="ExternalInput").ap()
    d_mask = nc.dram_tensor("maskbig", [128, 896], bf16, kind="ExternalInput").ap()
    d_id = nc.dram_tensor("ident", [128, 128], f32, kind="ExternalInput").ap()
    d_out = nc.dram_tensor("out", [NJ, NK, 128, TCOL], bf16,
                           kind="ExternalOutput").ap()

    from contextlib import ExitStack

    with tile.TileContext(nc) as tc, ExitStack() as stk:
        wp = stk.enter_context(tc.tile_pool(name="weights", bufs=1))
        xt_p = stk.enter_context(tc.tile_pool(name="xtp", bufs=2))
        ap_ = stk.enter_context(tc.tile_pool(name="acts", bufs=1))
        sq_p = stk.enter_context(tc.tile_pool(name="sq", bufs=2))
        es_p = stk.enter_context(tc.tile_pool(name="es", bufs=8))
        esm_p = stk.enter_context(tc.tile_pool(name="esum", bufs=2))
        rp = stk.enter_context(tc.tile_pool(name="rope", bufs=1))
        rd_p = stk.enter_context(tc.tile_pool(name="rdp", bufs=2))
        bc_p = stk.enter_context(tc.tile_pool(name="bcast", bufs=1))
        vt_p = stk.enter_context(tc.tile_pool(name="vtmp", bufs=2))
        o_p = stk.enter_context(tc.tile_pool(name="ocopy", bufs=1))
        dram_p = stk.enter_context(tc.tile_pool(name="dram", bufs=1, space="DRAM"))
        pp = stk.enter_context(tc.tile_pool(name="pp", bufs=2, space="PSUM"))
        ps_p = stk.enter_context(tc.tile_pool(name="ps", bufs=2, space="PSUM"))
        pa_p = stk.enter_context(tc.tile_pool(name="pa", bufs=2, space="PSUM"))
        pd_p = stk.enter_context(tc.tile_pool(name="pd", bufs=2, space="PSUM"))

        # ---- resident tiles ----
        ws1 = wp.tile([128, NK, 128], bf16)
        ws2 = wp.tile([128, NK, 64], bf16)
        wq = wp.tile([128, NK, 384], bf16)

        kvb = wp.tile([128, NL, 512], bf16)
        ow = wp.tile([128, HPC, HIDDEN], bf16)
        cosT = wp.tile([128, T], bf16)
        sinT = wp.tile([128, T], bf16)
        mswT = wp.tile([128, 128], bf16)
        maskb = wp.tile([128, 896], bf16)
        ident_f32 = wp.tile([128, 128], f32)
        ones = wp.tile([128, 1], bf16)
        # stage-A tensors share the xt stream ring (they are dead after
        # ~30us; ring deps serialize xt col 0/1 loads behind stage A reads)
        kva = xt_p.tile([128, NK, KV_RANK + D_ROPE], bf16, tag="xt",
                        name="kva")
        xta = xt_p.tile([128, NK, KV_RANK + D_ROPE], bf16, tag="xt",
                        name="xta")

        # activations (feature-major / transposed layouts)
        qn = [ap_.tile([128, T], bf16, tag=f"qn{h}", name=f"qn{h}")
              for h in range(HPC)]
        qpe = ap_.tile([128, T], bf16)          # h0 rows 0:64, h1 rows 64:128
        kn = [ap_.tile([128, T], bf16, tag=f"kn{h}", name=f"kn{h}")
              for h in range(HPC)]
        kpe = ap_.tile([128, T], bf16)          # duplicated in both 64-halves
        latT = ap_.tile([128, NL, T], bf16)
        vna = [ap_.tile([128, NS, D_V], bf16, tag=f"v{h}", name=f"v{h}")
               for h in range(HPC)]
        att = [ap_.tile([128, T], bf16, tag=f"att{h}", name=f"att{h}")
               for h in range(HPC)]
        lat_sb = ap_.tile([128, NL, TCA], bf16)
        ch4 = ap_.tile([128, TCA], bf16)
        ssqrow_q = ap_.tile([1, T], bf16)
        ssqa_q = ap_.tile([1, T], bf16)
        ssqkv = ap_.tile([1, T], bf16)
        srow_q = ap_.tile([1, T], f32)          # rsqrt'ed scales (row layout)
        srow_kv = ap_.tile([1, T], f32)
        skvcol_raw = ap_.tile([128, NS], bf16)
        skvcol = ap_.tile([128, NS], f32)

        # ---- initial DMA loads, most-urgent first per queue ----
        # stage A inputs lead both HWDGE queues, k-interleaved so the first
        # matmuls can start after ~1MB instead of the whole first wave
        dxta = d_xta.rearrange("k p t -> p k t")
        dkva = d_kva.rearrange("(k p) c -> p k c", p=128)
        for g in range(4):
            gs = slice(4 * g, 4 * g + 4)
            q = nc.sync if g < 2 else nc.scalar
            q.dma_start(kva[:, gs, :], dkva[:, gs, :])
            nc.sync.dma_start(xta[:, gs, 0:TCA], dxta[:, gs, :])
        nc.gpsimd.memset(ones[:], 1.0)
        # scalar queue: ssq weights, then stage-B weights + small tables
        dws1 = d_ws1.rearrange("(k p) c -> p k c", p=128)
        dws2 = d_ws2.rearrange("(k p) c -> p k c", p=128)
        for g in range(2):
            gs = slice(8 * g, 8 * g + 8)
            nc.scalar.dma_start(ws1[:, gs, :], dws1[:, gs, :])
            nc.scalar.dma_start(ws2[:, gs, :], dws2[:, gs, :])
        dwq = d_wq.rearrange("(k p) c -> p k c", p=128)
        for g in range(4):
            gs = slice(4 * g, 4 * g + 4)
            nc.scalar.dma_start(wq[:, gs, :], dwq[:, gs, :])
        nc.scalar.dma_start(cosT[:], d_cos[:])
        nc.scalar.dma_start(sinT[:], d_sin[:])
        nc.scalar.dma_start(mswT[:], d_msw[:])

        # ---- stage A: kv latent + kv-ssq for this core's 256 tokens ----
        dqa = pd_p.tile([1, TCOL], f32, tag="den", name="dqa")
        for g in range(NL):
            pl = pp.tile([128, TCOL], f32, tag="proj", name=f"latA{g}")
            for k in range(NK):
                nc.tensor.matmul(pl[:, 0:TCA], kva[:, k, 128 * g:128 * g + 128],
                                 xta[:, k, 0:TCA], start=(k == 0),
                                 stop=(k == NK - 1))
            sqa = sq_p.tile([128, TCOL], bf16, tag="sq", name=f"sqa{g}")
            nc.scalar.activation(sqa[:, 0:TCA], pl[:, 0:TCA], AF.Square)
            nc.vector.tensor_copy(lat_sb[:, g, :], pl[:, 0:TCA])
            nc.tensor.matmul(dqa[:, 0:TCA], ones[:, :], sqa[:, 0:TCA],
                             start=(g == 0), stop=(g == NL - 1),
                             skip_group_check=True)
            if g == NL - 1:
                nc.vector.tensor_copy(ch4[64:65, :], dqa[:, 0:TCA])
        plk = pp.tile([128, TCOL], f32, tag="proj", name="latApe")
        for k in range(NK):
            nc.tensor.matmul(plk[0:64, 0:TCA], kva[:, k, KV_RANK:],
                             xta[:, k, 0:TCA], start=(k == 0),
                             stop=(k == NK - 1))
        nc.vector.tensor_copy(ch4[0:64, :], plk[0:64, 0:TCA])

        cc_in = dram_p.tile([5, 128, TCA], bf16)
        cc_out = dram_p.tile([N_CORES, 5, 128, TCA], bf16, addr_space="Shared")
        nc.scalar.dma_start(cc_in[0:NL].rearrange("c p t -> p c t"), lat_sb[:])
        nc.scalar.dma_start(cc_in[NL], ch4[:])
        nc.gpsimd.collective_compute(
            "AllGather", Alu.bypass,
            replica_groups=[list(range(N_CORES))],
            ins=[cc_in.opt()], outs=[cc_out.opt()],
        )
        # ---- rope helper (src: AP of shape [rows, TCOL]) ----
        def rope(dst, src, rows, c):
            e = rp.tile([128, TCOL], f32, tag="re")
            f = rp.tile([128, TCOL], bf16, tag="rf")
            nc.vector.tensor_tensor(e[0:rows, :], src,
                                    cosT[0:rows, c], Alu.mult)
            nc.vector.tensor_tensor(f[0:rows, :], src,
                                    sinT[0:rows, c], Alu.mult)
            pr = ps_p.tile([128, TCOL], f32, tag="score")
            nc.tensor.matmul(pr[0:rows, :], mswT[0:rows, 0:rows], f[0:rows, :],
                             start=True, stop=True)
            nc.vector.tensor_tensor(dst[0:rows, c], e[0:rows, :], pr[0:rows, :],
                                    Alu.add)

        # ---- merged pass: q-ssq + fused q projections (single xt stream) ----
        ar_ins = [dram_p.tile([1, 2 * TCOL], bf16, name=f"ari{x}")
                  for x in range(2)]
        ar_outs = [dram_p.tile([1, 2 * TCOL], bf16, name=f"aro{x}")
                   for x in range(2)]
        for j in range(NJ):
            c = slice(TCOL * j, TCOL * (j + 1))
            xtj = xt_p.tile([128, NK, KV_RANK + D_ROPE], bf16, tag="xt",
                            name=f"xt{j}")
            for g in range(4):
                nc.sync.dma_start(
                    xtj[:, 4 * g:4 * g + 4, 0:TCOL],
                    d_xt[j, 4 * g:4 * g + 4].rearrange("k p t -> p k t"))
            # q-ssq shard
            p0 = pp.tile([128, TCOL], f32, tag="proj")
            for k in range(NK):
                nc.tensor.matmul(p0[:], ws1[:, k, :], xtj[:, k, 0:TCOL],
                                 start=(k == 0), stop=(k == NK - 1))
            s0 = sq_p.tile([128, TCOL], bf16, tag="sq")
            nc.scalar.activation(s0[:], p0[:], AF.Square)
            p1 = pp.tile([128, TCOL], f32, tag="proj")
            for k in range(NK):
                nc.tensor.matmul(p1[0:64, :], ws2[:, k, :], xtj[:, k, 0:TCOL],
                                 start=(k == 0), stop=(k == NK - 1))
            s1 = sq_p.tile([128, TCOL], bf16, tag="sq")
            nc.scalar.activation(s1[0:64, :], p1[0:64, :], AF.Square)
            dq = pd_p.tile([1, TCOL], f32, tag="den")
            nc.tensor.matmul(dq[:], ones[:, :], s0[:], start=True, stop=False)
            nc.tensor.matmul(dq[:], ones[0:64, :], s1[0:64, :],
                             start=False, stop=True)
            nc.vector.tensor_copy(ssqrow_q[0:1, c], dq[:])
            # fused q projections
            for h in range(HPC):
                p = pp.tile([128, TCOL], f32, tag="proj")
                for k in range(NK):
                    nc.tensor.matmul(p[:], wq[:, k, 128 * h:128 * h + 128],
                                     xtj[:, k, 0:TCOL],
                                     start=(k == 0), stop=(k == NK - 1))
                nc.vector.tensor_copy(qn[h][:, c], p[:])
            p = pp.tile([128, TCOL], f32, tag="proj")
            for k in range(NK):
                nc.tensor.matmul(p[:], wq[:, k, 256:384], xtj[:, k, 0:TCOL],
                                 start=(k == 0), stop=(k == NK - 1))
            rope(qpe, p[:, :], 128, c)
            # split q-ssq AllReduce: fire after columns 1 and 3
            if j in (1, 3):
                x = j // 2
                half = slice(TCOL * 2 * x, TCOL * 2 * (x + 1))
                nc.gpsimd.dma_start(ar_ins[x][:], ssqrow_q[0:1, half])
                nc.gpsimd.collective_compute(
                    "AllReduce", Alu.add,
                    replica_groups=[list(range(N_CORES))],
                    ins=[ar_ins[x].opt()], outs=[ar_outs[x].opt()],
                )
                nc.gpsimd.dma_start(ssqa_q[0:1, half], ar_outs[x][:])
                # q scale chain for this half
                nc.vector.tensor_scalar(srow_q[0:1, half], ssqa_q[0:1, half],
                                        1.0 / Q_RANK, EPS, Alu.mult, Alu.add)
                nc.vector.reciprocal_approx_fast(srow_q[0:1, half],
                                                 srow_q[0:1, half])
                nc.scalar.activation(srow_q[0:1, half], srow_q[0:1, half],
                                     AF.Sqrt)

        # deferred weights: not needed before ~85us, keep early HBM free
        nc.scalar.dma_start(kvb[:], d_kvb.rearrange("(k p) c -> p k c", p=128))
        nc.scalar.dma_start(ident_f32[:], d_id[:])
        nc.scalar.dma_start(maskb[:], d_mask[:])
        nc.scalar.dma_start(ow[:], d_ow.rearrange("(h p) c -> p h c", p=128))

        # unpack the gathered latent.  Emitted AFTER the merged pass so the
        # AllReduce trigger DMAs outrank these in queue priority; latT goes
        # over the sync HWDGE queue (idle once the xt stream is issued).
        for b in range(N_CORES):
            cb = slice(TCA * b, TCA * (b + 1))
            nc.sync.dma_start(latT[:, :, cb],
                              cc_out[b, 0:NL].rearrange("c p t -> p c t"))
            nc.sync.dma_start(kpe[0:64, cb], cc_out[b, NL, 0:64, :])
            nc.gpsimd.dma_start(ssqkv[0:1, cb], cc_out[b, NL, 64:65, :])
            nc.gpsimd.dma_start(
                skvcol_raw[:, 2 * b:2 * b + 2],
                cc_out[b, NL, 64:65, :].rearrange("o (g p) -> (o p) g", p=128))

        # kv scale chains (hoisted, full width)
        nc.vector.tensor_scalar(skvcol[:], skvcol_raw[:], 1.0 / KV_RANK, EPS,
                                Alu.mult, Alu.add)
        nc.vector.reciprocal_approx_fast(skvcol[:], skvcol[:])
        nc.scalar.activation(skvcol[:], skvcol[:], AF.Sqrt)
        nc.vector.tensor_scalar(srow_kv[:], ssqkv[:], 1.0 / KV_RANK, EPS,
                                Alu.mult, Alu.add)
        nc.vector.reciprocal_approx_fast(srow_kv[:], srow_kv[:])
        nc.scalar.activation(srow_kv[:], srow_kv[:], AF.Sqrt)

        # ---- stage B: kn + v from the gathered latent; kpe rope ----
        for j in range(NJ):
            c = slice(TCOL * j, TCOL * (j + 1))
            for h in range(HPC):
                pk = pp.tile([128, TCOL], f32, tag="proj", name=f"pkn{j}_{h}")
                for g in range(NL):
                    nc.tensor.matmul(pk[:], kvb[:, g, 128 * h:128 * h + 128],
                                     latT[:, g, c], start=(g == 0),
                                     stop=(g == NL - 1))
                nc.vector.tensor_copy(kn[h][:, c], pk[:])
            for h in range(HPC):
                pv = pp.tile([128, TCOL], f32, tag="proj", name=f"pvt{j}_{h}")
                for g in range(NL):
                    nc.tensor.matmul(pv[:],
                                     kvb[:, g, 256 + 128 * h:384 + 128 * h],
                                     latT[:, g, c], start=(g == 0),
                                     stop=(g == NL - 1))
                vts = vt_p.tile([128, TCOL], f32, tag="vt")
                nc.vector.tensor_copy(vts[:], pv[:])
                ptr = ps_p.tile([128, TCOL], f32, tag="score",
                                name=f"vtr{j}_{h}")
                for sl in range(4):
                    si = 4 * j + sl
                    nc.tensor.transpose(ptr[:, 128 * sl:128 * sl + 128],
                                        vts[:, 128 * sl:128 * sl + 128],
                                        ident_f32[:])
                    nc.vector.tensor_scalar_mul(
                        vna[h][:, si, :], ptr[:, 128 * sl:128 * sl + 128],
                        skvcol[:, si:si + 1])
            rope(kpe, kpe[0:64, c], 64, c)
            nc.sync.dma_start(kpe[64:128, c], kpe[0:64, c])

        # o_proj helpers: one m-tile at a time so emission can interleave
        # with the NEXT column's attention, filling PE dependency bubbles.
        # Output staged in 4-tile groups (ring of 2) to cap SBUF use.
        ostate = {}

        def emit_oproj_tile(jp, m):
            cp = slice(TCOL * jp, TCOL * (jp + 1))
            if m % 4 == 0:
                ostate['g'] = o_p.tile([128, 4, TCOL], bf16, tag="ot",
                                       name=f"ot{jp}_{m // 4}")
            # the last column's o_proj runs after attention has drained, so
            # the pa PSUM ring is free: alternate pools for a 4-deep ring
            pool = pa_p if (jp == NJ - 1 and m % 2 == 1) else pp
            tag = "attn" if pool is pa_p else "proj"
            po = pool.tile([128, TCOL], f32, tag=tag, name=f"po{jp}_{m}")
            for h in range(HPC):
                nc.tensor.matmul(po[:], ow[:, h, 128 * m:128 * m + 128],
                                 att[h][:, cp],
                                 start=(h == 0), stop=(h == HPC - 1))
            eng = nc.vector if m % 2 == 0 else nc.scalar
            if eng is nc.vector:
                eng.tensor_copy(ostate['g'][:, m % 4, :], po[:])
            else:
                nc.scalar.copy(ostate['g'][:, m % 4, :], po[:])
            if m % 4 == 3:
                doj = d_out[jp].rearrange("m p t -> p m t")
                nc.sync.dma_start(doj[:, m - 3:m + 1, :], ostate['g'][:])

        def emit_oproj_col(jp):
            for m in range(NK):
                emit_oproj_tile(jp, m)

        # ---- per-column: late scaling then attention for both heads ----
        for j in range(NJ):
            c = slice(TCOL * j, TCOL * (j + 1))
            sqB = bc_p.tile([128, TCOL], f32, tag="sqB")
            nc.gpsimd.partition_broadcast(sqB[:], srow_q[0:1, c])
            skvB = bc_p.tile([128, TCOL], f32, tag="skvB")
            nc.gpsimd.partition_broadcast(skvB[:], srow_kv[0:1, c])
            for h in range(HPC):
                nc.vector.tensor_tensor(qn[h][:, c], qn[h][:, c], sqB[:],
                                        Alu.mult)
                nc.vector.tensor_tensor(kn[h][:, c], kn[h][:, c], skvB[:],
                                        Alu.mult)
            nc.vector.tensor_tensor(qpe[:, c], qpe[:, c], sqB[:], Alu.mult)

            # attention in S^T[s, t] layout, causal block-skip, software-
            # pipelined emission (den/PV of step i-1 after scores of step i)
            pa2 = [pa_p.tile([128, TCOL], f32, tag="attn", name=f"pa{j}_{h}")
                   for h in range(HPC)]
            pden2 = [pd_p.tile([1, TCOL], f32, tag="den", name=f"pden{j}_{h}")
                     for h in range(HPC)]
            n_s = 4 * (j + 1)
            es_hist = [[], []]
            om = NK
            if j > 0:
                om = 0
                # warm-up filler for the PE while the scale mults run
                while om < 4:
                    emit_oproj_tile(j - 1, om)
                    om += 1

            def emit_pv(i, h):
                nc.tensor.matmul(pa2[h][:], vna[h][:, i, :], es_hist[h][i][:],
                                 start=(i == 0), stop=(i == n_s - 1),
                                 skip_group_check=True)

            def emit_den_pair(i2, h):
                # den over an es pair pre-summed on DVE: halves the PE's
                # ones-matmul traffic; emitted 2 steps late so the DVE add
                # never heads-of-line-blocks the PE queue
                esum = esm_p.tile([128, TCOL], bf16, tag="esum")
                nc.vector.tensor_tensor(esum[:], es_hist[h][i2 - 1][:],
                                        es_hist[h][i2][:], Alu.add)
                nc.tensor.matmul(pden2[h][:], ones[:, :], esum[:],
                                 start=(i2 == 1), stop=(i2 == n_s - 1),
                                 skip_group_check=True)

            for i in range(n_s):
                for h in range(HPC):
                    st = ps_p.tile([128, TCOL], f32, tag="score")
                    nc.tensor.matmul(st[:], kn[h][:, 128 * i:128 * i + 128],
                                     qn[h][:, c], start=True, stop=False)
                    nc.tensor.matmul(st[:],
                                     kpe[64 * h:64 * h + 64,
                                         128 * i:128 * i + 128],
                                     qpe[64 * h:64 * h + 64, c],
                                     start=False, stop=True)
                    if i > 0:
                        emit_pv(i - 1, h)
                    if i >= 3 and (i - 2) % 2 == 1:
                        emit_den_pair(i - 2, h)
                    if i >= 4 * j:
                        ko = i - 4 * j
                        nc.vector.tensor_tensor(
                            st[:], st[:],
                            maskb[:, 384 - 128 * ko:896 - 128 * ko], Alu.add)
                    es = es_p.tile([128, TCOL], bf16, tag="es")
                    nc.scalar.activation(es[:], st[:], AF.Exp)
                    es_hist[h].append(es)
                # spread previous column's o_proj through this column's
                # attention as ready PE work
                while om < min(NK, (i + 1) * NK // n_s + 2):
                    emit_oproj_tile(j - 1, om)
                    om += 1
            while om < NK:
                emit_oproj_tile(j - 1, om)
                om += 1
            for h in range(HPC):
                emit_pv(n_s - 1, h)
            for h in range(HPC):
                emit_den_pair(n_s - 1, h)
            for h in range(HPC):
                rden = rd_p.tile([1, TCOL], f32, tag="rden")
                nc.vector.reciprocal_approx_fast(rden[:], pden2[h][:])
                rdB = rd_p.tile([128, TCOL], f32, tag="rdB")
                nc.gpsimd.partition_broadcast(rdB[:], rden[:])
                nc.vector.tensor_tensor(att[h][:, c], pa2[h][:], rdB[:],
                                        Alu.mult)

        # o_proj for the last column (earlier columns were emitted inside
        # the following column's attention loop as PE filler work)
        emit_oproj_col(NJ - 1)

    nc.compile()
    return nc


def _host_prep(positions, hidden_states, q_a_w, q_a_ln_w, q_b_w,
               kv_a_w, kv_a_ln_w, kv_b_w, o_w):
    pos = np.asarray(positions, dtype=np.float32)
    hs = np.asarray(hidden_states, dtype=np.float32)
    q_a_w = np.asarray(q_a_w, dtype=np.float32)
    q_b_w = np.asarray(q_b_w, dtype=np.float32) * np.asarray(
        q_a_ln_w, dtype=np.float32)[:, None]
    kv_a_w = np.asarray(kv_a_w, dtype=np.float32)
    kv_b_w = np.asarray(kv_b_w, dtype=np.float32) * np.asarray(
        kv_a_ln_w, dtype=np.float32)[:, None]
    o_w = np.asarray(o_w, dtype=np.float32)

    # fused q weights (softmax scale folded in)
    wq_full = (q_a_w @ q_b_w).reshape(HIDDEN, H, QK_DIM) * SCALE
    kvb = kv_b_w.reshape(KV_RANK, H, D_NOPE + D_V)

    # rope pair permutation: interleaved (0::2, 1::2) -> (x1 block | x2 block)
    qpe_cols = wq_full[:, :, D_NOPE:]
    qpe_perm = np.concatenate([qpe_cols[:, :, 0::2], qpe_cols[:, :, 1::2]],
                              axis=2)  # [HIDDEN, H, 64]
    kva_perm = kv_a_w.copy()
    wkpe = kv_a_w[:, KV_RANK:]
    kva_perm[:, KV_RANK:] = np.concatenate([wkpe[:, 0::2], wkpe[:, 1::2]],
                                           axis=1)

    inv_freq = 1.0 / (ROPE_BASE ** (np.arange(0, D_ROPE, 2,
                                              dtype=np.float32) / D_ROPE))
    freqs = pos[None, :] * inv_freq[:, None]           # [32, T]
    cosT = np.tile(np.cos(freqs), (4, 1)).astype(BF16)   # [128, T]
    sinT = np.tile(np.sin(freqs), (4, 1)).astype(BF16)

    # band-swap-with-sign matrix: o = e + Msw @ f
    msw = np.zeros((128, 128), dtype=np.float32)
    for q in range(2):
        for i in range(32):
            msw[64 * q + i, 64 * q + 32 + i] = -1.0
            msw[64 * q + 32 + i, 64 * q + i] = 1.0
    mswT = np.ascontiguousarray(msw.T).astype(BF16)

    # big causal mask: maskb[s, col] = 0 if col >= s + 384 else NEG
    col = np.arange(896)[None, :]
    s_ = np.arange(128)[:, None]
    maskb = np.where(col >= s_ + 384, 0.0, NEG).astype(BF16)

    hsT = np.ascontiguousarray(hs.T).astype(BF16)      # [HIDDEN, T]
    xt = np.ascontiguousarray(
        hsT.reshape(NK, 128, NJ, TCOL).transpose(2, 0, 1, 3))
    xta_all = hsT.reshape(NK, 128, N_CORES, TCA)

    ident = np.eye(128, dtype=np.float32)

    in_maps = []
    for cidx in range(N_CORES):
        h0 = HPC * cidx
        wq_c = np.concatenate(
            [wq_full[:, h0 + h, :D_NOPE] for h in range(HPC)]
            + [qpe_perm[:, h0 + h, :] for h in range(HPC)], axis=1)
        kvb_c = np.concatenate(
            [kvb[:, h0 + h, :D_NOPE] for h in range(HPC)]
            + [kvb[:, h0 + h, D_NOPE:] for h in range(HPC)], axis=1)
        ws1 = q_a_w[:, 192 * cidx:192 * cidx + 128]
        ws2 = q_a_w[:, 192 * cidx + 128:192 * (cidx + 1)]
        ow_c = o_w[D_V * h0:D_V * (h0 + HPC), :]
        in_maps.append({
            "xt": xt,
            "xta": np.ascontiguousarray(xta_all[:, :, cidx, :]),
            "wq": np.ascontiguousarray(wq_c).astype(BF16),
            "kva": np.ascontiguousarray(kva_perm).astype(BF16),
            "kvb": np.ascontiguousarray(kvb_c).astype(BF16),
            "wssq1": np.ascontiguousarray(ws1).astype(BF16),
            "wssq2": np.ascontiguousarray(ws2).astype(BF16),
            "ow": np.ascontiguousarray(ow_c).astype(BF16),
            "cosT": cosT,
            "sinT": sinT,
            "mswT": mswT,
            "maskbig": maskb,
            "ident": ident,
        })
    return in_maps


def kernel(**inputs):
    from concourse.bass_utils import run_bass_kernel_spmd

    if "nc" not in _CACHE:
        _CACHE["nc"] = _build_program()
    nc = _CACHE["nc"]

    in_maps = _host_prep(**inputs)
    trace = bool(int(os.environ.get("BASSK_TRACE", "0")))
    tmpdir = os.environ.get("BASSK_TMPDIR") or None
    if tmpdir:
        os.makedirs(tmpdir, exist_ok=True)
    res = run_bass_kernel_spmd(nc, in_maps, core_ids=list(range(N_CORES)),
                               trace=trace, tmpdir=tmpdir)
    _CACHE["last_exec_time_ns"] = res.exec_time_ns
    _CACHE["last_results"] = res.results
    outT = np.zeros((NJ, NK, 128, TCOL), dtype=np.float32)
    for r in res.results:
        outT += np.asarray(r["out"], dtype=np.float32)
    outT = outT.transpose(1, 2, 0, 3).reshape(HIDDEN, T)
    return np.ascontiguousarray(outT.T)
